# revision 8
# baseline (speedup 1.0000x reference)
"""Self-contained Trainium2 Bass kernel for nn_GNN_75436805587134.

kernel(**inputs) -> np.ndarray [1024, 1]

Strategy: dst-sharded message passing across 8 NeuronCores; bf16-replicated
node-state table updated via CHUNKED AllGathers overlapped with compute;
4-bank dma_gather for h[src] rows; edge aggregation as one-hot S-tile
matmuls built ON-CHIP (iota + is_equal); LayerNorm via bn_stats + batched
Newton rsqrt (no activation-table thrash); residual kept fp32 in SBUF;
input featurization + graph pooling + head on host.
"""
import sys
for _p in ("/opt/trn_rl_repo",):
    if _p not in sys.path:
        sys.path.insert(0, _p)
import numpy as np
import ml_dtypes

import concourse.bass as bass
import concourse.bacc as bacc
import concourse.tile as tile
import concourse.mybir as mybir
import concourse.bass_utils as bass_utils

bf16 = ml_dtypes.bfloat16


N, E, B = 100000, 400000, 1024
NF, EF, H, C, D = 32, 16, 256, 256, 6
LN_EPS = 1e-5
NC = 8
RPC = N // NC                # 12500 real rows per core
NT = 98                      # dst tiles per core (98*128 = 12544)
TR = NT * 128                # 12544 padded rows per core
TROWS = NC * TR              # 100352 table rows
BANKS = 4
BANK = TROWS // BANKS        # 25088
WLEN = 4                     # dst-tiles per window
NW = (NT + WLEN - 1) // WLEN  # 25 windows
MAX_TILES_PER_CALL = 16
GRP = 8                      # tiles per DMA flush group
NGRP = (NT + GRP - 1) // GRP  # 13
CHUNK_T = [0, 24, 48, 72, 98]  # AllGather chunk boundaries (tile idx)


CHUNK_T_ARR = np.array(CHUNK_T)


def trow_of(v):
    """global node id -> padded table row (chunk-major layout).

    Table layout: [chunk c][core k][rows of chunk c], so that each chunked
    AllGather (in: agin rows of chunk c; out: concat over cores) writes one
    contiguous region.
    """
    v = np.asarray(v)
    k = v // RPC
    r = v - k * RPC                      # padded local row, < TR
    t = r // 128
    c = np.searchsorted(CHUNK_T_ARR, t, side="right") - 1
    base = NC * 128 * CHUNK_T_ARR[c]
    rows_c = (CHUNK_T_ARR[c + 1] - CHUNK_T_ARR[c]) * 128
    return base + k * rows_c + (r - CHUNK_T_ARR[c] * 128)


def build_edge_structure(src, dst):
    """Returns uniform SPMD structure + per-core tile data."""
    src = np.asarray(src).astype(np.int64)
    dst = np.asarray(dst).astype(np.int64)
    deg_out = np.maximum(np.bincount(src, minlength=N), 1.0)
    deg_in = np.maximum(np.bincount(dst, minlength=N), 1.0)
    no = deg_out ** -0.5
    ni = deg_in ** -0.5
    w_edge = (no[src] * ni[dst]).astype(np.float32)
    ratio = (1.0 / deg_in[dst] / w_edge).astype(np.float32)  # w2/w per edge

    trow = trow_of(src)
    bank = trow // BANK
    brow = trow % BANK

    core = dst // RPC
    dloc = dst - core * RPC
    t_of_e = dloc // 128
    drel = dloc - t_of_e * 128

    # counts per (core, t, bank)
    cnt = np.zeros((NC, NT, BANKS), np.int64)
    np.add.at(cnt, (core, t_of_e, bank), 1)
    ntiles_tb = np.maximum(np.ceil(cnt / 128).astype(np.int64).max(axis=0), 1)  # [NT, BANKS]

    # tile order: for w: for b: for t in w: range(ntiles_tb[t, b])
    tile_t = []      # target dst-tile per tile
    tile_b = []
    call_list = []   # (bank, tile_start, n_tiles) uniform
    for w in range(NW):
        ts = range(w * WLEN, min((w + 1) * WLEN, NT))
        for b in range(BANKS):
            run_start = len(tile_t)
            for t in ts:
                for i in range(ntiles_tb[t, b]):
                    tile_t.append(t)
                    tile_b.append(b)
            n = len(tile_t) - run_start
            s = run_start
            while n > 0:
                c = min(n, MAX_TILES_PER_CALL)
                call_list.append((b, s, c))
                s += c
                n -= c
    ntot = len(tile_t)
    tile_t = np.array(tile_t)
    tile_b = np.array(tile_b)

    # per-core per-tile data
    # order edges of core k by (t, bank, dloc)
    gidx = np.zeros((NC, ntot, 128), np.int16)
    sval = np.zeros((NC, ntot, 128), np.float32)   # folded w (0 for pads)
    srel = np.zeros((NC, ntot, 128), np.int16)     # dst col in S tile

    # tile slot index for (t, b): starting tile index
    tile_start_of = {}
    for i, (t, b) in enumerate(zip(tile_t, tile_b)):
        tile_start_of.setdefault((t, b), i)

    order = np.lexsort((drel, bank, t_of_e, core))
    e_sorted = order
    key_core = core[order]
    key_t = t_of_e[order]
    key_b = bank[order]
    # group boundaries
    import itertools
    for (k, t, b), grp in itertools.groupby(
            range(len(order)), key=lambda i: (key_core[i], key_t[i], key_b[i])):
        grp = list(grp)
        eids = e_sorted[grp]
        base = tile_start_of[(t, b)]
        for j, e in enumerate(eids):
            ti = base + j // 128
            jj = j % 128
            gidx[k, ti, jj] = brow[e]
            sval[k, ti, jj] = w_edge[e]
            srel[k, ti, jj] = drel[e]
    return dict(ntiles_tb=ntiles_tb, tile_t=tile_t, tile_b=tile_b,
                call_list=call_list, ntot=ntot,
                gidx=gidx, sval=sval, srel=srel,
                no=no, ni=ni, deg_in=deg_in, ratio=ratio)


def wrap_idx(idx_tiles):
    """[ntot,128] int16 -> [128, ntot*8] wrapped+replicated layout"""
    ntot = idx_tiles.shape[0]
    out = np.zeros((128, ntot * 8), np.int16)
    for ti in range(ntot):
        w = idx_tiles[ti].reshape(8, 16).T  # [16, 8]
        out[:, ti * 8:(ti + 1) * 8] = np.tile(w, (8, 1))
    return out


def prep_weights(inp):
    """Fold LN gammas into film tables; build device weight arrays."""
    g1 = np.asarray(inp["ln2_g"], np.float32)    # layer path uses ln2/film2/conv2
    b1 = np.asarray(inp["ln2_b"], np.float32)
    fw = np.asarray(inp["film2_w"], np.float32)  # [D, C, 2H]
    fb = np.asarray(inp["film2_b"], np.float32)  # [D, 2H]
    film_tab = np.zeros((D, C + 1, 2 * H), np.float32)
    for i in range(D):
        gam_w = fw[i, :, :H]; bet_w = fw[i, :, H:]
        gam_b = fb[i, :H]; bet_b = fb[i, H:]
        gl = g1[i]; bl = b1[i]
        # Gamma_eff = gl*(1+gamma);  Beta_eff = bl*(1+gamma) + beta
        film_tab[i, :C, :H] = gam_w * gl[None, :]
        film_tab[i, C, :H] = gl * (1.0 + gam_b)
        film_tab[i, :C, H:] = bet_w + gam_w * bl[None, :]
        film_tab[i, C, H:] = bet_b + bl * (1.0 + gam_b)
    return film_tab


def sincos_emb(t):
    half = 64
    freqs = np.exp(-np.log(1000.0) * np.arange(half, dtype=np.float32) / half)
    a = (np.asarray(t, np.float32) * 1000.0)[:, None] * freqs[None, :]
    return np.concatenate([np.sin(a), np.cos(a)], axis=-1).astype(np.float32)  # [B,128]


def host_pool_head(h6_full_real, n_index, head_w, head_b):
    """h6_full_real: [N, H] f32 (real rows). Returns [B, 1]."""
    n_index = np.asarray(n_index).astype(np.int64)
    cnt = np.maximum(np.bincount(n_index, minlength=B), 1.0)
    pooled = np.zeros((B, H), np.float64)
    np.add.at(pooled, n_index, h6_full_real.astype(np.float64))
    g_mean = (pooled / cnt[:, None]).astype(np.float32)
    return g_mean @ np.asarray(head_w, np.float32) + np.asarray(head_b, np.float32)


def host_input_aggregates(inp, st):
    """Exact f32 input-stage aggregation per core -> [NC][64, TR] bf16 (transposed)."""
    src = np.asarray(inp["src"]).astype(np.int64)
    dst = np.asarray(inp["dst"]).astype(np.int64)
    w_e = (st["no"][src] * st["ni"][dst]).astype(np.float32)
    nx = np.asarray(inp["node_x"], np.float32)
    agg_x = np.zeros((N, NF), np.float32)
    np.add.at(agg_x, dst, w_e[:, None] * nx[src])
    ratio = st["ratio"]
    ee = np.asarray(inp["edge_e"], np.float32) * ratio[:, None]
    e_aug = np.concatenate([ee, ratio[:, None]], 1)
    agg_e = np.zeros((N, 17), np.float32)
    np.add.at(agg_e, dst, w_e[:, None] * e_aug)
    outs = []
    for k in range(NC):
        a = np.zeros((64, TR), np.float32)
        a[0:NF, :RPC] = agg_x[k * RPC:(k + 1) * RPC].T
        a[32:49, :RPC] = agg_e[k * RPC:(k + 1) * RPC].T
        outs.append(a.astype(bf16))
    return outs


def kernel(**inputs):
    out, _res, _h6 = run(inputs, trace=False)
    return out.astype(np.float32)


dt = mybir.dt
AF = mybir.ActivationFunctionType
ALU = mybir.AluOpType
NCORES = NC
MAGIC = 0x5f3759df


def build(st, nlayers=6):
    ntot = st["ntot"]
    tile_t = st["tile_t"]
    call_list = st["call_list"]

    nc = bacc.Bacc("TRN2", target_bir_lowering=False, debug=False,
                   enable_asserts=False, num_devices=NCORES, num_swdge_queues=4)

    # ---------- I/O ----------
    svq = nc.dram_tensor("svq", [128, ntot * 2], dt.float32, kind="ExternalInput").ap()
    gidx = nc.dram_tensor("gidx", [128, ntot * 8], dt.int16, kind="ExternalInput").ap()
    nidx = nc.dram_tensor("nidx", [128, NT * 8], dt.int16, kind="ExternalInput").ap()
    embT = nc.dram_tensor("embT", [128, B], dt.bfloat16, kind="ExternalInput").ap()
    w1 = nc.dram_tensor("w1", [128, 256], dt.bfloat16, kind="ExternalInput").ap()
    w2 = nc.dram_tensor("w2", [256, 256], dt.bfloat16, kind="ExternalInput").ap()
    tb1 = nc.dram_tensor("tb1", [128, 2], dt.float32, kind="ExternalInput").ap()
    tb2 = nc.dram_tensor("tb2", [128, 2], dt.float32, kind="ExternalInput").ap()
    film = nc.dram_tensor("film", [6 * 257, 512], dt.bfloat16, kind="ExternalInput").ap()
    wconv = nc.dram_tensor("wconv", [6 * 256, 256], dt.bfloat16, kind="ExternalInput").ap()
    bconv = nc.dram_tensor("bconv", [6, 256], dt.bfloat16, kind="ExternalInput").ap()
    win_w = nc.dram_tensor("win_w", [32, 256], dt.bfloat16, kind="ExternalInput").ap()
    we_w = nc.dram_tensor("we_w", [32, 256], dt.bfloat16, kind="ExternalInput").ap()
    inb = nc.dram_tensor("inb", [1, 256], dt.bfloat16, kind="ExternalInput").ap()
    axe = nc.dram_tensor("axe", [64, TR], dt.bfloat16, kind="ExternalInput").ap()
    h6_out = nc.dram_tensor("h6_out", [TR, 256], dt.float32, kind="ExternalOutput").ap()

    # ---------- internal DRAM ----------
    agin = [nc.dram_tensor(f"agin{i}", [TR, 256], dt.bfloat16, kind="Internal").ap()
            for i in range(2)]
    tabs = [nc.dram_tensor(f"tab{i}", [TROWS, 256], dt.bfloat16, kind="Internal",
                           addr_space="Shared").ap() for i in range(2)]
    gbd = nc.dram_tensor("gbd", [6 * B, 512], dt.bfloat16, kind="Internal").ap()

    RG = [list(range(NCORES))]

    def windows_of():
        out = []
        ci = 0
        for w in range(NW):
            ts = list(range(w * WLEN, min((w + 1) * WLEN, NT)))
            calls = []
            while ci < len(call_list):
                b, a, n = call_list[ci]
                if tile_t[a] // WLEN != w:
                    break
                calls.append((b, a, n))
                ci += 1
            out.append((w, ts, calls))
        return out

    WINS = windows_of()
    # chunk id of a tile
    def chunk_of(t):
        for c in range(4):
            if t < CHUNK_T[c + 1]:
                return c
        return 3

    def ag_chunk(nc, src, dstb, c):
        # chunk-major table: AG chunk c writes the contiguous region
        # [NC*r0, NC*r1) of the table (concat over cores of chunk-c rows).
        r0, r1 = CHUNK_T[c] * 128, CHUNK_T[c + 1] * 128
        in_ap = src[r0:r1, :]
        out_ap = dstb[NCORES * r0:NCORES * r1, :]
        nc.gpsimd.collective_compute(
            "AllGather", ALU.bypass, replica_groups=RG,
            ins=[in_ap.opt()], outs=[out_ap.opt()])

    with tile.TileContext(nc) as tc:
        with tc.tile_pool(name="const", bufs=1) as constp, \
             tc.tile_pool(name="resp", bufs=1) as resp, \
             tc.tile_pool(name="wpool", bufs=2) as wpool, \
             tc.tile_pool(name="svp", bufs=2) as svp, \
             tc.tile_pool(name="gpool", bufs=2) as gpool, \
             tc.tile_pool(name="spool", bufs=2) as spool, \
             tc.tile_pool(name="gbpool", bufs=2) as gbpool, \
             tc.tile_pool(name="agst", bufs=2) as agstp, \
             tc.tile_pool(name="work", bufs=3) as work, \
             tc.tile_pool(name="tiny", bufs=3) as tiny, \
             tc.tile_pool(name="psZ", bufs=5, space="PSUM") as psZ:

            # ---- resident constants ----
            gidx_sb = constp.tile([128, ntot * 8], dt.int16)
            nc.sync.dma_start(gidx_sb[:], gidx[:])
            nidx_sb = constp.tile([128, NT * 8], dt.int16)
            nc.sync.dma_start(nidx_sb[:], nidx[:])
            ones1 = constp.tile([1, 128], dt.bfloat16)
            nc.vector.memset(ones1[:], 1.0)
            win_sb = constp.tile([32, 256], dt.bfloat16)
            nc.sync.dma_start(win_sb[:], win_w[:])
            we_sb = constp.tile([32, 256], dt.bfloat16)
            nc.sync.dma_start(we_sb[:], we_w[:])
            inb_sb = constp.tile([1, 256], dt.bfloat16)
            nc.sync.dma_start(inb_sb[:], inb[:])
            iota_i = constp.tile([128, 128], dt.int32)
            nc.gpsimd.iota(iota_i[:], [[1, 128]], base=0, channel_multiplier=0)
            iota_f = constp.tile([128, 128], dt.float32)
            nc.gpsimd.tensor_copy(iota_f[:], iota_i[:])

            # residual state, fp32, resident in SBUF
            res = resp.tile([128, NT, 256], dt.float32)

            # ---- phase B: input stage (aggregates precomputed on host) ----
            with tc.tile_pool(name="axp", bufs=2) as axp:
                for g in range(NGRP):
                    t0g = g * GRP
                    ntl = min(GRP, NT - t0g)
                    ax_sb = axp.tile([32, GRP * 128], dt.bfloat16, tag="axw")
                    nc.sync.dma_start(ax_sb[:, 0:ntl * 128],
                                      axe[0:32, t0g * 128:(t0g + ntl) * 128])
                    ae_sb = axp.tile([32, GRP * 128], dt.bfloat16, tag="aew")
                    nc.sync.dma_start(ae_sb[:, 0:ntl * 128],
                                      axe[32:64, t0g * 128:(t0g + ntl) * 128])
                    ag = agstp.tile([128, GRP, 256], dt.bfloat16, tag="ag")
                    for i in range(ntl):
                        t = t0g + i
                        zps = psZ.tile([128, 256], dt.float32, tag="z", space="PSUM")
                        nc.tensor.matmul(zps[:], ax_sb[:, i * 128:(i + 1) * 128],
                                         win_sb[:32, :], start=True, stop=False)
                        nc.tensor.matmul(zps[:], ae_sb[:, i * 128:(i + 1) * 128],
                                         we_sb[:32, :], start=False, stop=False)
                        nc.tensor.matmul(zps[:], ones1[:], inb_sb[:], start=False, stop=True)
                        nc.scalar.activation(res[:, t, :], zps[:], AF.Copy)
                        nc.vector.tensor_copy(ag[:, i, :], zps[:])
                    nc.sync.dma_start(
                        agin[0].rearrange("(t p) f -> p t f", p=128)[:, t0g:t0g + ntl, :],
                        ag[:, 0:ntl, :])
                    # fire AG chunks as their groups complete
                    for c in range(4):
                        if CHUNK_T[c + 1] == t0g + ntl:
                            ag_chunk(nc, agin[0], tabs[0], c)

            # ---- phase A: cond -> gamma/beta tables (overlaps phase B DMA) ----
            with tc.tile_pool(name="condp", bufs=1) as condp, \
                 tc.tile_pool(name="psC", bufs=2, space="PSUM") as psC:
                embT_sb = condp.tile([128, B], dt.bfloat16)
                nc.sync.dma_start(embT_sb[:], embT[:])
                w1_sb = condp.tile([128, 256], dt.bfloat16)
                nc.sync.dma_start(w1_sb[:], w1[:])
                w2_sb = condp.tile([128, 2, 256], dt.bfloat16)
                nc.sync.dma_start(w2_sb[:], w2[:].rearrange("(a p) n -> p a n", p=128))
                tb1_sb = condp.tile([128, 2], dt.float32)
                nc.sync.dma_start(tb1_sb[:], tb1[:])
                tb2_sb = condp.tile([128, 2], dt.float32)
                nc.sync.dma_start(tb2_sb[:], tb2[:])

                c1T = condp.tile([128, 2, B], dt.bfloat16)  # [part, oc, t]
                for oc in range(2):
                    for tb in range(2):
                        ps = psC.tile([128, 512], dt.float32, tag="condps", space="PSUM")
                        nc.tensor.matmul(ps[:], w1_sb[:, oc * 128:(oc + 1) * 128],
                                         embT_sb[:, tb * 512:(tb + 1) * 512],
                                         start=True, stop=True)
                        nc.scalar.activation(c1T[:, oc, tb * 512:(tb + 1) * 512], ps[:],
                                             AF.Silu, bias=tb1_sb[:, oc:oc + 1])
                c2T = condp.tile([128, 2, B], dt.bfloat16)
                for oc in range(2):
                    for tb in range(2):
                        ps = psC.tile([128, 512], dt.float32, tag="condps", space="PSUM")
                        for k in range(2):
                            nc.tensor.matmul(ps[:], w2_sb[:, k, oc * 128:(oc + 1) * 128],
                                             c1T[:, k, tb * 512:(tb + 1) * 512],
                                             start=(k == 0), stop=(k == 1))
                        nc.scalar.activation(c2T[:, oc, tb * 512:(tb + 1) * 512], ps[:],
                                             AF.Identity, bias=tb2_sb[:, oc:oc + 1])
                for l in range(6):
                    film_sb = wpool.tile([128, 2, 512], dt.bfloat16, tag="film")
                    nc.sync.dma_start(film_sb[:], film[l * 257:l * 257 + 256, :]
                                      .rearrange("(a p) n -> p a n", p=128))
                    filmb_sb = wpool.tile([1, 512], dt.bfloat16, tag="filmb")
                    nc.sync.dma_start(filmb_sb[:], film[l * 257 + 256:l * 257 + 257, :])
                    for gc in range(8):
                        ps = psC.tile([128, 512], dt.float32, tag="condps", space="PSUM")
                        for k in range(2):
                            nc.tensor.matmul(ps[:], c2T[:, k, gc * 128:(gc + 1) * 128],
                                             film_sb[:, k, :], start=(k == 0), stop=False)
                        nc.tensor.matmul(ps[:], ones1[:], filmb_sb[:],
                                         start=False, stop=True)
                        gbt = tiny.tile([128, 512], dt.bfloat16, tag="gbt")
                        nc.vector.tensor_copy(gbt[:], ps[:])
                        nc.sync.dma_start(
                            gbd[(l * B + gc * 128):(l * B + (gc + 1) * 128), :], gbt[:])

            # ---- phase C: layers ----
            with tc.tile_pool(name="psAgg", bufs=1, space="PSUM") as psAgg:
                for l in range(nlayers):
                    table = tabs[l % 2]
                    last = l == nlayers - 1

                    wl_sb = wpool.tile([128, 2, 256], dt.bfloat16, tag="wl")
                    nc.sync.dma_start(wl_sb[:], wconv[l * 256:(l + 1) * 256, :]
                                      .rearrange("(a p) n -> p a n", p=128))
                    bl_sb = wpool.tile([1, 256], dt.bfloat16, tag="bl")
                    nc.sync.dma_start(bl_sb[:], bconv[l:l + 1, :])
                    sv_sb = svp.tile([128, ntot, 2], dt.float32, tag="sv")
                    nc.sync.dma_start(sv_sb[:], svq[:].rearrange("p (t c) -> p t c", c=2))

                    gb_tiles = {}
                    for cidx in range(13):
                        t0c = cidx * 8
                        ntl = min(8, NT - t0c)
                        gbg = gbpool.tile([128, 8, 512], dt.bfloat16, tag="gbg")
                        nc.gpsimd.dma_gather(
                            gbg[:, 0:ntl, :], gbd[l * B:(l + 1) * B, :],
                            nidx_sb[:, t0c * 8:(t0c + ntl) * 8], ntl * 128, ntl * 128, 512,
                            queue_num=cidx % 4)
                        for i in range(ntl):
                            gb_tiles[t0c + i] = (gbg, i)

                    qi = 0
                    ag = None
                    for (w, ts, calls) in WINS:
                        alo = psAgg.tile([128, 512], dt.float32, tag="alo", space="PSUM")
                        ahi = psAgg.tile([128, 512], dt.float32, tag="ahi", space="PSUM")
                        w_first = calls[0][1]
                        w_last = calls[-1][1] + calls[-1][2] - 1
                        for (b, a, n) in calls:
                            g = gpool.tile([128, MAX_TILES_PER_CALL, 256], dt.bfloat16, tag="g")
                            nc.gpsimd.dma_gather(
                                g[:, 0:n, :], table[b * BANK:(b + 1) * BANK, :],
                                gidx_sb[:, a * 8:(a + n) * 8], n * 128, n * 128, 256,
                                queue_num=qi % 4)
                            qi += 1
                            s_sb = spool.tile([128, MAX_TILES_PER_CALL * 128], dt.bfloat16,
                                              tag="s")
                            for i in range(n):
                                ti = a + i
                                nc.gpsimd.tensor_scalar(
                                    out=s_sb[:, i * 128:(i + 1) * 128], in0=iota_f[:],
                                    scalar1=sv_sb[:, ti, 0:1], scalar2=sv_sb[:, ti, 1:2],
                                    op0=ALU.is_equal, op1=ALU.mult)
                            for i in range(n):
                                ti = a + i
                                seg = int(tile_t[ti]) % WLEN
                                st_f = ti == w_first
                                sp_f = ti == w_last
                                nc.tensor.matmul(alo[:, seg * 128:(seg + 1) * 128],
                                                 g[:, i, 0:128],
                                                 s_sb[:, i * 128:(i + 1) * 128],
                                                 start=st_f, stop=sp_f)
                                nc.tensor.matmul(ahi[:, seg * 128:(seg + 1) * 128],
                                                 g[:, i, 128:256],
                                                 s_sb[:, i * 128:(i + 1) * 128],
                                                 start=st_f, stop=sp_f)
                        # post-process the window's 4 dst tiles
                        zpss = []
                        st6 = tiny.tile([128, WLEN, 6], dt.float32, tag="st6")
                        mv = tiny.tile([128, WLEN, 2], dt.float32, tag="mv")
                        for j, t in enumerate(ts):
                            seg = t % WLEN
                            ab_lo = work.tile([128, 128], dt.bfloat16, tag="ablo")
                            nc.scalar.activation(ab_lo[:], alo[:, seg * 128:(seg + 1) * 128],
                                                 AF.Copy)
                            ab_hi = work.tile([128, 128], dt.bfloat16, tag="abhi")
                            nc.scalar.activation(ab_hi[:], ahi[:, seg * 128:(seg + 1) * 128],
                                                 AF.Copy)
                            zps = psZ.tile([128, 256], dt.float32, tag="z", space="PSUM")
                            nc.tensor.matmul(zps[:], ab_lo[:], wl_sb[:, 0, :], start=True, stop=False)
                            nc.tensor.matmul(zps[:], ab_hi[:], wl_sb[:, 1, :], start=False, stop=False)
                            nc.tensor.matmul(zps[:], ones1[:], bl_sb[:], start=False, stop=True)
                            zpss.append(zps)
                            nc.vector.bn_stats(st6[:, j, :], zps[:])
                            nc.vector.bn_aggr(mv[:, j, :], st6[:, j, :])
                        nw_ = len(ts)
                        # batched LN scale: negmu, rstd = -mean, rsqrt(var+eps)
                        negmu = tiny.tile([128, WLEN], dt.float32, tag="negmu")
                        nc.vector.tensor_scalar_mul(negmu[:, 0:nw_], mv[:, 0:nw_, 0], -1.0)
                        veps = tiny.tile([128, WLEN], dt.float32, tag="veps")
                        nc.vector.tensor_scalar_add(veps[:, 0:nw_], mv[:, 0:nw_, 1], LN_EPS)
                        yi = tiny.tile([128, WLEN], dt.int32, tag="yi")
                        nc.vector.tensor_scalar(out=yi[:, 0:nw_],
                                                in0=veps[:, 0:nw_].bitcast(dt.int32),
                                                scalar1=1, scalar2=None,
                                                op0=ALU.arith_shift_right)
                        nc.vector.tensor_scalar(out=yi[:, 0:nw_], in0=yi[:, 0:nw_],
                                                scalar1=-1, scalar2=MAGIC,
                                                op0=ALU.mult, op1=ALU.add)
                        y0 = yi[:, 0:nw_].bitcast(dt.float32)
                        rstd = tiny.tile([128, WLEN], dt.float32, tag="rstd")
                        tnw = tiny.tile([128, WLEN], dt.float32, tag="tnw")
                        # two Newton iterations: y = y*(1.5 - 0.5*v*y*y)
                        nc.vector.tensor_tensor(out=tnw[:, 0:nw_], in0=veps[:, 0:nw_], in1=y0, op=ALU.mult)
                        nc.vector.tensor_tensor(out=tnw[:, 0:nw_], in0=tnw[:, 0:nw_], in1=y0, op=ALU.mult)
                        nc.vector.tensor_scalar(out=tnw[:, 0:nw_], in0=tnw[:, 0:nw_],
                                                scalar1=-0.5, scalar2=1.5, op0=ALU.mult, op1=ALU.add)
                        nc.vector.tensor_tensor(out=rstd[:, 0:nw_], in0=y0, in1=tnw[:, 0:nw_], op=ALU.mult)
                        nc.vector.tensor_tensor(out=tnw[:, 0:nw_], in0=veps[:, 0:nw_], in1=rstd[:, 0:nw_], op=ALU.mult)
                        nc.vector.tensor_tensor(out=tnw[:, 0:nw_], in0=tnw[:, 0:nw_], in1=rstd[:, 0:nw_], op=ALU.mult)
                        nc.vector.tensor_scalar(out=tnw[:, 0:nw_], in0=tnw[:, 0:nw_],
                                                scalar1=-0.5, scalar2=1.5, op0=ALU.mult, op1=ALU.add)
                        nc.vector.tensor_tensor(out=rstd[:, 0:nw_], in0=rstd[:, 0:nw_], in1=tnw[:, 0:nw_], op=ALU.mult)

                        for j, t in enumerate(ts):
                            zps = zpss[j]
                            xh = work.tile([128, 256], dt.bfloat16, tag="xh")
                            nc.vector.tensor_scalar(out=xh[:], in0=zps[:],
                                                    scalar1=negmu[:, j:j + 1],
                                                    scalar2=rstd[:, j:j + 1],
                                                    op0=ALU.add, op1=ALU.mult)
                            gbg, gi = gb_tiles[t]
                            y = work.tile([128, 256], dt.bfloat16, tag="y")
                            nc.vector.tensor_tensor(out=y[:], in0=xh[:], in1=gbg[:, gi, 0:256], op=ALU.mult)
                            nc.vector.tensor_tensor(out=y[:], in0=y[:], in1=gbg[:, gi, 256:512], op=ALU.add)
                            h2 = work.tile([128, 256], dt.float32, tag="h2")
                            nc.scalar.activation(h2[:], y[:], AF.Silu)
                            nc.vector.tensor_tensor(out=res[:, t, :], in0=h2[:],
                                                    in1=res[:, t, :], op=ALU.add)
                            if not last:
                                ig = t - (t // GRP) * GRP
                                if ig == 0:
                                    ag = agstp.tile([128, GRP, 256], dt.bfloat16, tag="ag")
                                nc.gpsimd.tensor_copy(ag[:, ig, :], res[:, t, :])
                                if t == NT - 1 or ig == GRP - 1:
                                    t0g = (t // GRP) * GRP
                                    ntl = t - t0g + 1
                                    nc.sync.dma_start(
                                        agin[(l + 1) % 2].rearrange(
                                            "(t p) f -> p t f", p=128)[:, t0g:t0g + ntl, :],
                                        ag[:, 0:ntl, :])
                                for c in range(4):
                                    if CHUNK_T[c + 1] == t + 1:
                                        ag_chunk(nc, agin[(l + 1) % 2], tabs[(l + 1) % 2], c)

                # final output: residual SBUF -> DRAM
                nc.sync.dma_start(
                    h6_out.rearrange("(t p) f -> p t f", p=128), res[:, :, :])

    nc.compile()
    return nc


def make_inputs(inp, st):
    """Build per-core in_maps."""
    n_index = np.asarray(inp["n_index"]).astype(np.int64)
    ntot = st["ntot"]

    emb = sincos_emb(inp["t"])
    embT = np.ascontiguousarray(emb.T).astype(bf16)

    film_tab = prep_weights(inp)
    film_dev = film_tab.reshape(6 * 257, 512).astype(bf16)
    wconv = np.asarray(inp["conv2_w"], np.float32).reshape(6 * 256, 256).astype(bf16)
    bconv = np.asarray(inp["conv2_b"], np.float32).astype(bf16)

    w_in32 = np.zeros((32, 256), bf16)
    w_in32[:NF] = np.asarray(inp["in_conv_w"], np.float32).astype(bf16)
    we32 = np.zeros((32, 256), bf16)
    we32[:EF] = np.asarray(inp["edge_w"], np.float32).astype(bf16)
    we32[EF] = np.asarray(inp["edge_b"], np.float32).astype(bf16)
    inb = np.asarray(inp["in_conv_b"], np.float32).astype(bf16)[None, :]

    tb1 = np.ascontiguousarray(np.asarray(inp["t_b1"], np.float32).reshape(2, 128).T)
    tb2 = np.ascontiguousarray(np.asarray(inp["t_b2"], np.float32).reshape(2, 128).T)
    w1 = np.asarray(inp["t_w1"], np.float32).astype(bf16)
    w2 = np.asarray(inp["t_w2"], np.float32).astype(bf16)

    axes = host_input_aggregates(inp, st)

    in_maps = []
    for k in range(NCORES):
        # svq: [128, ntot, 2] f32 -> [128, ntot*2]
        sv = np.zeros((128, ntot, 2), np.float32)
        sv[:, :, 0] = st["srel"][k].T.astype(np.float32)
        sv[:, :, 1] = st["sval"][k].T
        sv_dev = np.ascontiguousarray(sv.reshape(128, ntot * 2))
        gidx_dev = wrap_idx(st["gidx"][k])
        gvals = np.zeros(TR, np.int16)
        gvals[:RPC] = n_index[k * RPC:(k + 1) * RPC].astype(np.int16)
        nidx_dev = wrap_idx(gvals.reshape(NT, 128))
        in_maps.append({
            "svq": sv_dev, "gidx": gidx_dev,
            "nidx": nidx_dev, "embT": embT, "w1": w1, "w2": w2,
            "tb1": tb1, "tb2": tb2, "film": film_dev, "wconv": wconv,
            "bconv": bconv, "win_w": w_in32, "we_w": we32, "inb": inb,
            "axe": axes[k],
        })
    return in_maps


def run(inp, trace=False, nlayers=6):
    src = np.asarray(inp["src"]).astype(np.int64)
    dst = np.asarray(inp["dst"]).astype(np.int64)
    st = build_edge_structure(src, dst)
    nc = build(st, nlayers=nlayers)
    in_maps = make_inputs(inp, st)
    res = bass_utils.run_bass_kernel_spmd(
        nc, in_maps, core_ids=list(range(NCORES)), trace=trace,
        trace_cores=[0] if trace else None)
    h6 = np.concatenate([res.results[k]["h6_out"][:RPC] for k in range(NCORES)], 0)
    out = host_pool_head(h6, np.asarray(inp["n_index"]), inp["head_w"], inp["head_b"])
    return out, res, h6


# revision 18
# speedup vs baseline: 1.9383x; 1.9383x over previous
"""Self-contained Trainium2 Bass kernel for nn_GNN_75436805587134.

kernel(**inputs) -> np.ndarray [1024, 1]

Strategy: dst-sharded message passing across 8 NeuronCores; bf16-replicated
node-state table updated via CHUNKED AllGathers overlapped with compute;
4-bank dma_gather for h[src] rows; edge aggregation as one-hot S-tile
matmuls built ON-CHIP (iota + is_equal); LayerNorm via bn_stats + batched
Newton rsqrt (no activation-table thrash); residual kept fp32 in SBUF;
input featurization + graph pooling + head on host.
"""
import sys
for _p in ("/opt/trn_rl_repo",):
    if _p not in sys.path:
        sys.path.insert(0, _p)
import numpy as np
import ml_dtypes

import concourse.bass as bass
import concourse.bacc as bacc
import concourse.tile as tile
import concourse.mybir as mybir
import concourse.bass_utils as bass_utils

bf16 = ml_dtypes.bfloat16


N, E, B = 100000, 400000, 1024
NF, EF, H, C, D = 32, 16, 256, 256, 6
LN_EPS = 1e-5
NC = 8
RPC = N // NC                # 12500 real rows per core
NT = 98                      # dst tiles per core (98*128 = 12544)
TR = NT * 128                # 12544 padded rows per core
TROWS = NC * TR              # 100352 table rows
BANKS = 4
BANK = TROWS // BANKS        # 25088
WLEN = 4                     # dst-tiles per window
NW = (NT + WLEN - 1) // WLEN  # 25 windows
MAX_TILES_PER_CALL = 16
GRP = 8                      # tiles per DMA flush group
NGRP = (NT + GRP - 1) // GRP  # 13
CHUNK_T = [0, 24, 48, 72, 98]  # AllGather chunk boundaries (tile idx)


CHUNK_T_ARR = np.array(CHUNK_T)


def trow_of(v):
    """global node id -> padded table row (chunk-major layout).

    Table layout: [chunk c][core k][rows of chunk c], so that each chunked
    AllGather (in: agin rows of chunk c; out: concat over cores) writes one
    contiguous region.
    """
    v = np.asarray(v)
    k = v // RPC
    r = v - k * RPC                      # padded local row, < TR
    t = r // 128
    c = np.searchsorted(CHUNK_T_ARR, t, side="right") - 1
    base = NC * 128 * CHUNK_T_ARR[c]
    rows_c = (CHUNK_T_ARR[c + 1] - CHUNK_T_ARR[c]) * 128
    return base + k * rows_c + (r - CHUNK_T_ARR[c] * 128)


def build_edge_structure(src, dst):
    """Returns uniform SPMD structure + per-core tile data."""
    src = np.asarray(src).astype(np.int64)
    dst = np.asarray(dst).astype(np.int64)
    deg_out = np.maximum(np.bincount(src, minlength=N), 1.0)
    deg_in = np.maximum(np.bincount(dst, minlength=N), 1.0)
    no = deg_out ** -0.5
    ni = deg_in ** -0.5
    w_edge = (no[src] * ni[dst]).astype(np.float32)
    ratio = (1.0 / deg_in[dst] / w_edge).astype(np.float32)  # w2/w per edge

    trow = trow_of(src)
    bank = trow // BANK
    brow = trow % BANK

    core = dst // RPC
    dloc = dst - core * RPC
    t_of_e = dloc // 128
    drel = dloc - t_of_e * 128

    # counts per (core, t, bank)
    cnt = np.zeros((NC, NT, BANKS), np.int64)
    np.add.at(cnt, (core, t_of_e, bank), 1)
    ntiles_tb = np.maximum(np.ceil(cnt / 128).astype(np.int64).max(axis=0), 1)  # [NT, BANKS]

    # tile order: for w: for b: for t in w: range(ntiles_tb[t, b])
    tile_t = []      # target dst-tile per tile
    tile_b = []
    call_list = []   # (bank, tile_start, n_tiles) uniform
    for w in range(NW):
        ts = range(w * WLEN, min((w + 1) * WLEN, NT))
        for b in range(BANKS):
            run_start = len(tile_t)
            for t in ts:
                for i in range(ntiles_tb[t, b]):
                    tile_t.append(t)
                    tile_b.append(b)
            n = len(tile_t) - run_start
            s = run_start
            while n > 0:
                c = min(n, MAX_TILES_PER_CALL)
                call_list.append((b, s, c))
                s += c
                n -= c
    ntot = len(tile_t)
    tile_t = np.array(tile_t)
    tile_b = np.array(tile_b)

    # per-core per-tile data
    # order edges of core k by (t, bank, dloc)
    gidx = np.zeros((NC, ntot, 128), np.int16)
    sval = np.zeros((NC, ntot, 128), np.float32)   # folded w (0 for pads)
    srel = np.zeros((NC, ntot, 128), np.int16)     # dst col in S tile

    # tile slot index for (t, b): starting tile index
    tile_start_of = {}
    for i, (t, b) in enumerate(zip(tile_t, tile_b)):
        tile_start_of.setdefault((t, b), i)

    order = np.lexsort((drel, bank, t_of_e, core))
    e_sorted = order
    key_core = core[order]
    key_t = t_of_e[order]
    key_b = bank[order]
    # group boundaries
    import itertools
    for (k, t, b), grp in itertools.groupby(
            range(len(order)), key=lambda i: (key_core[i], key_t[i], key_b[i])):
        grp = list(grp)
        eids = e_sorted[grp]
        base = tile_start_of[(t, b)]
        for j, e in enumerate(eids):
            ti = base + j // 128
            jj = j % 128
            gidx[k, ti, jj] = brow[e]
            sval[k, ti, jj] = w_edge[e]
            srel[k, ti, jj] = drel[e]
    return dict(ntiles_tb=ntiles_tb, tile_t=tile_t, tile_b=tile_b,
                call_list=call_list, ntot=ntot,
                gidx=gidx, sval=sval, srel=srel,
                no=no, ni=ni, deg_in=deg_in, ratio=ratio)


def build_S(st):
    """[NC, ntot, 128, 128] bf16 one-hot*weight"""
    NCn, ntot = st["sval"].shape[0], st["ntot"]
    S = np.zeros((NCn, ntot, 128, 128), bf16)
    j = np.arange(128)
    for k in range(NCn):
        for ti in range(ntot):
            S[k, ti, j, st["srel"][k, ti]] = st["sval"][k, ti].astype(bf16)
            # pad rows have sval 0 -> harmless entry at col srel=0
    return S


def wrap_idx(idx_tiles):
    """[ntot,128] int16 -> [128, ntot*8] wrapped+replicated layout"""
    ntot = idx_tiles.shape[0]
    out = np.zeros((128, ntot * 8), np.int16)
    for ti in range(ntot):
        w = idx_tiles[ti].reshape(8, 16).T  # [16, 8]
        out[:, ti * 8:(ti + 1) * 8] = np.tile(w, (8, 1))
    return out


def prep_weights(inp):
    """Fold LN gammas into film tables; build device weight arrays."""
    g1 = np.asarray(inp["ln2_g"], np.float32)    # layer path uses ln2/film2/conv2
    b1 = np.asarray(inp["ln2_b"], np.float32)
    fw = np.asarray(inp["film2_w"], np.float32)  # [D, C, 2H]
    fb = np.asarray(inp["film2_b"], np.float32)  # [D, 2H]
    film_tab = np.zeros((D, C + 1, 2 * H), np.float32)
    for i in range(D):
        gam_w = fw[i, :, :H]; bet_w = fw[i, :, H:]
        gam_b = fb[i, :H]; bet_b = fb[i, H:]
        gl = g1[i]; bl = b1[i]
        # Gamma_eff = gl*(1+gamma);  Beta_eff = bl*(1+gamma) + beta
        film_tab[i, :C, :H] = gam_w * gl[None, :]
        film_tab[i, C, :H] = gl * (1.0 + gam_b)
        film_tab[i, :C, H:] = bet_w + gam_w * bl[None, :]
        film_tab[i, C, H:] = bet_b + bl * (1.0 + gam_b)
    return film_tab


def sincos_emb(t):
    half = 64
    freqs = np.exp(-np.log(1000.0) * np.arange(half, dtype=np.float32) / half)
    a = (np.asarray(t, np.float32) * 1000.0)[:, None] * freqs[None, :]
    return np.concatenate([np.sin(a), np.cos(a)], axis=-1).astype(np.float32)  # [B,128]


def host_pool_head(h6_full_real, n_index, head_w, head_b):
    """h6_full_real: [N, H] f32 (real rows). Returns [B, 1]."""
    n_index = np.asarray(n_index).astype(np.int64)
    cnt = np.maximum(np.bincount(n_index, minlength=B), 1.0)
    pooled = np.zeros((B, H), np.float64)
    np.add.at(pooled, n_index, h6_full_real.astype(np.float64))
    g_mean = (pooled / cnt[:, None]).astype(np.float32)
    return g_mean @ np.asarray(head_w, np.float32) + np.asarray(head_b, np.float32)


def host_input_aggregates(inp, st):
    """Exact f32 input-stage aggregation per core -> [NC][64, TR] bf16 (transposed)."""
    src = np.asarray(inp["src"]).astype(np.int64)
    dst = np.asarray(inp["dst"]).astype(np.int64)
    w_e = (st["no"][src] * st["ni"][dst]).astype(np.float32)
    nx = np.asarray(inp["node_x"], np.float32)
    agg_x = np.zeros((N, NF), np.float32)
    np.add.at(agg_x, dst, w_e[:, None] * nx[src])
    ratio = st["ratio"]
    ee = np.asarray(inp["edge_e"], np.float32) * ratio[:, None]
    e_aug = np.concatenate([ee, ratio[:, None]], 1)
    agg_e = np.zeros((N, 17), np.float32)
    np.add.at(agg_e, dst, w_e[:, None] * e_aug)
    outs = []
    for k in range(NC):
        a = np.zeros((64, TR), np.float32)
        a[0:NF, :RPC] = agg_x[k * RPC:(k + 1) * RPC].T
        a[32:49, :RPC] = agg_e[k * RPC:(k + 1) * RPC].T
        outs.append(a.astype(bf16))
    return outs


def kernel(**inputs):
    out, _res, _h6 = run(inputs, trace=False)
    return out.astype(np.float32)


dt = mybir.dt
AF = mybir.ActivationFunctionType
ALU = mybir.AluOpType
NCORES = NC
MAGIC = 0x5f3759df


def build(st, nlayers=6):
    ntot = st["ntot"]
    tile_t = st["tile_t"]
    call_list = st["call_list"]

    nc = bacc.Bacc("TRN2", target_bir_lowering=False, debug=False,
                   enable_asserts=False, num_devices=NCORES, num_swdge_queues=4)

    # ---------- I/O ----------
    S_in = nc.dram_tensor("S_in", [128, ntot * 128], dt.bfloat16, kind="ExternalInput").ap()
    gidx = nc.dram_tensor("gidx", [128, ntot * 8], dt.int16, kind="ExternalInput").ap()
    nidx = nc.dram_tensor("nidx", [128, NT * 8], dt.int16, kind="ExternalInput").ap()
    embT = nc.dram_tensor("embT", [128, B], dt.bfloat16, kind="ExternalInput").ap()
    w1 = nc.dram_tensor("w1", [128, 256], dt.bfloat16, kind="ExternalInput").ap()
    w2 = nc.dram_tensor("w2", [256, 256], dt.bfloat16, kind="ExternalInput").ap()
    tb1 = nc.dram_tensor("tb1", [128, 2], dt.float32, kind="ExternalInput").ap()
    tb2 = nc.dram_tensor("tb2", [128, 2], dt.float32, kind="ExternalInput").ap()
    film = nc.dram_tensor("film", [6 * 257, 512], dt.bfloat16, kind="ExternalInput").ap()
    wconv = nc.dram_tensor("wconv", [6 * 256, 256], dt.bfloat16, kind="ExternalInput").ap()
    bconv = nc.dram_tensor("bconv", [6, 256], dt.bfloat16, kind="ExternalInput").ap()
    win_w = nc.dram_tensor("win_w", [32, 256], dt.bfloat16, kind="ExternalInput").ap()
    we_w = nc.dram_tensor("we_w", [32, 256], dt.bfloat16, kind="ExternalInput").ap()
    inb = nc.dram_tensor("inb", [1, 256], dt.bfloat16, kind="ExternalInput").ap()
    axe = nc.dram_tensor("axe", [64, TR], dt.bfloat16, kind="ExternalInput").ap()
    h6_out = nc.dram_tensor("h6_out", [TR, 256], dt.float32, kind="ExternalOutput").ap()

    # ---------- internal DRAM ----------
    agin = [nc.dram_tensor(f"agin{i}", [TR, 256], dt.bfloat16, kind="Internal").ap()
            for i in range(2)]
    tabs = [nc.dram_tensor(f"tab{i}", [TROWS, 256], dt.bfloat16, kind="Internal",
                           addr_space="Shared").ap() for i in range(2)]
    gbd = nc.dram_tensor("gbd", [6 * B, 512], dt.bfloat16, kind="Internal").ap()

    RG = [list(range(NCORES))]

    def windows_of():
        out = []
        ci = 0
        for w in range(NW):
            ts = list(range(w * WLEN, min((w + 1) * WLEN, NT)))
            calls = []
            while ci < len(call_list):
                b, a, n = call_list[ci]
                if tile_t[a] // WLEN != w:
                    break
                calls.append((b, a, n))
                ci += 1
            out.append((w, ts, calls))
        return out

    WINS = windows_of()
    # chunk id of a tile
    def chunk_of(t):
        for c in range(4):
            if t < CHUNK_T[c + 1]:
                return c
        return 3

    def ag_chunk(nc, src, dstb, c):
        # chunk-major table: AG chunk c writes the contiguous region
        # [NC*r0, NC*r1) of the table (concat over cores of chunk-c rows).
        r0, r1 = CHUNK_T[c] * 128, CHUNK_T[c + 1] * 128
        in_ap = src[r0:r1, :]
        out_ap = dstb[NCORES * r0:NCORES * r1, :]
        nc.gpsimd.collective_compute(
            "AllGather", ALU.bypass, replica_groups=RG,
            ins=[in_ap.opt()], outs=[out_ap.opt()])

    with tile.TileContext(nc) as tc:
        with tc.tile_pool(name="const", bufs=1) as constp, \
             tc.tile_pool(name="resp", bufs=1) as resp, \
             tc.tile_pool(name="wpool", bufs=2) as wpool, \
             tc.tile_pool(name="gpool", bufs=2) as gpool, \
             tc.tile_pool(name="spool", bufs=2) as spool, \
             tc.tile_pool(name="gbpool", bufs=2) as gbpool, \
             tc.tile_pool(name="agst", bufs=2) as agstp, \
             tc.tile_pool(name="work", bufs=3) as work, \
             tc.tile_pool(name="tiny", bufs=3) as tiny, \
             tc.tile_pool(name="psZ", bufs=5, space="PSUM") as psZ:

            # ---- resident constants ----
            gidx_sb = constp.tile([128, ntot * 8], dt.int16)
            nc.sync.dma_start(gidx_sb[:], gidx[:])
            nidx_sb = constp.tile([128, NT * 8], dt.int16)
            nc.sync.dma_start(nidx_sb[:], nidx[:])
            ones1 = constp.tile([1, 128], dt.bfloat16)
            nc.vector.memset(ones1[:], 1.0)
            win_sb = constp.tile([32, 256], dt.bfloat16)
            nc.sync.dma_start(win_sb[:], win_w[:])
            we_sb = constp.tile([32, 256], dt.bfloat16)
            nc.sync.dma_start(we_sb[:], we_w[:])
            inb_sb = constp.tile([1, 256], dt.bfloat16)
            nc.sync.dma_start(inb_sb[:], inb[:])

            # residual state, fp32, resident in SBUF
            res = resp.tile([128, NT, 256], dt.float32)

            # ---- phase B: input stage (aggregates precomputed on host) ----
            with tc.tile_pool(name="axp", bufs=2) as axp:
                for g in range(NGRP):
                    t0g = g * GRP
                    ntl = min(GRP, NT - t0g)
                    ax_sb = axp.tile([32, GRP * 128], dt.bfloat16, tag="axw")
                    nc.sync.dma_start(ax_sb[:, 0:ntl * 128],
                                      axe[0:32, t0g * 128:(t0g + ntl) * 128])
                    ae_sb = axp.tile([32, GRP * 128], dt.bfloat16, tag="aew")
                    nc.sync.dma_start(ae_sb[:, 0:ntl * 128],
                                      axe[32:64, t0g * 128:(t0g + ntl) * 128])
                    ag = agstp.tile([128, GRP, 256], dt.bfloat16, tag="ag")
                    for i in range(ntl):
                        t = t0g + i
                        zps = psZ.tile([128, 256], dt.float32, tag="z", space="PSUM")
                        nc.tensor.matmul(zps[:], ax_sb[:, i * 128:(i + 1) * 128],
                                         win_sb[:32, :], start=True, stop=False)
                        nc.tensor.matmul(zps[:], ae_sb[:, i * 128:(i + 1) * 128],
                                         we_sb[:32, :], start=False, stop=False)
                        nc.tensor.matmul(zps[:], ones1[:], inb_sb[:], start=False, stop=True)
                        nc.scalar.activation(res[:, t, :], zps[:], AF.Copy)
                        nc.vector.tensor_copy(ag[:, i, :], zps[:])
                    nc.sync.dma_start(
                        agin[0].rearrange("(t p) f -> p t f", p=128)[:, t0g:t0g + ntl, :],
                        ag[:, 0:ntl, :])
                    # fire AG chunks as their groups complete
                    for c in range(4):
                        if CHUNK_T[c + 1] == t0g + ntl:
                            ag_chunk(nc, agin[0], tabs[0], c)

            # ---- phase A: cond -> gamma/beta tables (overlaps phase B DMA) ----
            with tc.tile_pool(name="condp", bufs=1) as condp, \
                 tc.tile_pool(name="psC", bufs=2, space="PSUM") as psC:
                embT_sb = condp.tile([128, B], dt.bfloat16)
                nc.sync.dma_start(embT_sb[:], embT[:])
                w1_sb = condp.tile([128, 256], dt.bfloat16)
                nc.sync.dma_start(w1_sb[:], w1[:])
                w2_sb = condp.tile([128, 2, 256], dt.bfloat16)
                nc.sync.dma_start(w2_sb[:], w2[:].rearrange("(a p) n -> p a n", p=128))
                tb1_sb = condp.tile([128, 2], dt.float32)
                nc.sync.dma_start(tb1_sb[:], tb1[:])
                tb2_sb = condp.tile([128, 2], dt.float32)
                nc.sync.dma_start(tb2_sb[:], tb2[:])

                c1T = condp.tile([128, 2, B], dt.bfloat16)  # [part, oc, t]
                for oc in range(2):
                    for tb in range(2):
                        ps = psC.tile([128, 512], dt.float32, tag="condps", space="PSUM")
                        nc.tensor.matmul(ps[:], w1_sb[:, oc * 128:(oc + 1) * 128],
                                         embT_sb[:, tb * 512:(tb + 1) * 512],
                                         start=True, stop=True)
                        nc.scalar.activation(c1T[:, oc, tb * 512:(tb + 1) * 512], ps[:],
                                             AF.Silu, bias=tb1_sb[:, oc:oc + 1])
                c2T = condp.tile([128, 2, B], dt.bfloat16)
                for oc in range(2):
                    for tb in range(2):
                        ps = psC.tile([128, 512], dt.float32, tag="condps", space="PSUM")
                        for k in range(2):
                            nc.tensor.matmul(ps[:], w2_sb[:, k, oc * 128:(oc + 1) * 128],
                                             c1T[:, k, tb * 512:(tb + 1) * 512],
                                             start=(k == 0), stop=(k == 1))
                        nc.scalar.activation(c2T[:, oc, tb * 512:(tb + 1) * 512], ps[:],
                                             AF.Identity, bias=tb2_sb[:, oc:oc + 1])
                for l in range(6):
                    film_sb = wpool.tile([128, 2, 512], dt.bfloat16, tag="film")
                    nc.sync.dma_start(film_sb[:], film[l * 257:l * 257 + 256, :]
                                      .rearrange("(a p) n -> p a n", p=128))
                    filmb_sb = wpool.tile([1, 512], dt.bfloat16, tag="filmb")
                    nc.sync.dma_start(filmb_sb[:], film[l * 257 + 256:l * 257 + 257, :])
                    for gc in range(8):
                        ps = psC.tile([128, 512], dt.float32, tag="condps", space="PSUM")
                        for k in range(2):
                            nc.tensor.matmul(ps[:], c2T[:, k, gc * 128:(gc + 1) * 128],
                                             film_sb[:, k, :], start=(k == 0), stop=False)
                        nc.tensor.matmul(ps[:], ones1[:], filmb_sb[:],
                                         start=False, stop=True)
                        gbt = tiny.tile([128, 512], dt.bfloat16, tag="gbt")
                        nc.vector.tensor_copy(gbt[:], ps[:])
                        nc.sync.dma_start(
                            gbd[(l * B + gc * 128):(l * B + (gc + 1) * 128), :], gbt[:])

            # ---- phase C: layers ----
            with tc.tile_pool(name="psAgg", bufs=1, space="PSUM") as psAgg:
                for l in range(nlayers):
                    table = tabs[l % 2]
                    last = l == nlayers - 1

                    wl_sb = wpool.tile([128, 2, 256], dt.bfloat16, tag="wl")
                    nc.sync.dma_start(wl_sb[:], wconv[l * 256:(l + 1) * 256, :]
                                      .rearrange("(a p) n -> p a n", p=128))
                    bl_sb = wpool.tile([1, 256], dt.bfloat16, tag="bl")
                    nc.sync.dma_start(bl_sb[:], bconv[l:l + 1, :])

                    gb_tiles = {}
                    for cidx in range(13):
                        t0c = cidx * 8
                        ntl = min(8, NT - t0c)
                        gbg = gbpool.tile([128, 8, 512], dt.bfloat16, tag="gbg")
                        nc.gpsimd.dma_gather(
                            gbg[:, 0:ntl, :], gbd[l * B:(l + 1) * B, :],
                            nidx_sb[:, t0c * 8:(t0c + ntl) * 8], ntl * 128, ntl * 128, 512,
                            queue_num=cidx % 4)
                        for i in range(ntl):
                            gb_tiles[t0c + i] = (gbg, i)

                    qi = 0
                    ag = None
                    for (w, ts, calls) in WINS:
                        alo = psAgg.tile([128, 512], dt.float32, tag="alo", space="PSUM")
                        ahi = psAgg.tile([128, 512], dt.float32, tag="ahi", space="PSUM")
                        w_first = calls[0][1]
                        w_last = calls[-1][1] + calls[-1][2] - 1
                        w_ntiles = w_last - w_first + 1
                        s_sb = spool.tile([128, 40 * 128], dt.bfloat16, tag="s")
                        nc.sync.dma_start(s_sb[:, 0:w_ntiles * 128],
                                          S_in[:, w_first * 128:(w_last + 1) * 128])
                        for (b, a, n) in calls:
                            g = gpool.tile([128, MAX_TILES_PER_CALL, 256], dt.bfloat16, tag="g")
                            nc.gpsimd.dma_gather(
                                g[:, 0:n, :], table[b * BANK:(b + 1) * BANK, :],
                                gidx_sb[:, a * 8:(a + n) * 8], n * 128, n * 128, 256,
                                queue_num=qi % 4)
                            qi += 1
                            for i in range(n):
                                ti = a + i
                                seg = int(tile_t[ti]) % WLEN
                                st_f = ti == w_first
                                sp_f = ti == w_last
                                sl = ti - w_first
                                nc.tensor.matmul(alo[:, seg * 128:(seg + 1) * 128],
                                                 g[:, i, 0:128],
                                                 s_sb[:, sl * 128:(sl + 1) * 128],
                                                 start=st_f, stop=sp_f)
                                nc.tensor.matmul(ahi[:, seg * 128:(seg + 1) * 128],
                                                 g[:, i, 128:256],
                                                 s_sb[:, sl * 128:(sl + 1) * 128],
                                                 start=st_f, stop=sp_f)
                        # post-process the window's 4 dst tiles
                        zpss = []
                        st6 = tiny.tile([128, WLEN * 6], dt.float32, tag="st6")
                        mvt = tiny.tile([128, WLEN * 2], dt.float32, tag="mvt")
                        for j, t in enumerate(ts):
                            seg = t % WLEN
                            ab_lo = work.tile([128, 128], dt.bfloat16, tag="ablo")
                            nc.scalar.activation(ab_lo[:], alo[:, seg * 128:(seg + 1) * 128],
                                                 AF.Copy)
                            ab_hi = work.tile([128, 128], dt.bfloat16, tag="abhi")
                            nc.scalar.activation(ab_hi[:], ahi[:, seg * 128:(seg + 1) * 128],
                                                 AF.Copy)
                            zps = psZ.tile([128, 256], dt.float32, tag="z", space="PSUM")
                            nc.tensor.matmul(zps[:], ab_lo[:], wl_sb[:, 0, :], start=True, stop=False)
                            nc.tensor.matmul(zps[:], ab_hi[:], wl_sb[:, 1, :], start=False, stop=False)
                            nc.tensor.matmul(zps[:], ones1[:], bl_sb[:], start=False, stop=True)
                            zpss.append(zps)
                            nc.vector.bn_stats(st6[:, j * 6:(j + 1) * 6], zps[:])
                            nc.vector.bn_aggr(mvt[:, j * 2:(j + 1) * 2], st6[:, j * 6:(j + 1) * 6])
                        # full-tile LN scale: rsqrt(var+eps) via bit-trick + 2 Newton
                        # iterations run on the interleaved (mean,var) tile; the
                        # rsqrt of the mean columns is computed but unused.
                        vepst = tiny.tile([128, WLEN * 2], dt.float32, tag="vepst")
                        nc.vector.tensor_scalar_add(vepst[:], mvt[:], LN_EPS)
                        negmvt = tiny.tile([128, WLEN * 2], dt.float32, tag="negmvt")
                        nc.vector.tensor_scalar_mul(negmvt[:], mvt[:], -1.0)
                        yi = tiny.tile([128, WLEN * 2], dt.int32, tag="yi")
                        nc.vector.tensor_scalar(out=yi[:], in0=vepst[:].bitcast(dt.int32),
                                                scalar1=1, scalar2=None,
                                                op0=ALU.arith_shift_right)
                        nc.vector.tensor_scalar(out=yi[:], in0=yi[:],
                                                scalar1=-1, scalar2=MAGIC,
                                                op0=ALU.mult, op1=ALU.add)
                        y0 = yi[:].bitcast(dt.float32)
                        rstd = tiny.tile([128, WLEN * 2], dt.float32, tag="rstd")
                        tnw = tiny.tile([128, WLEN * 2], dt.float32, tag="tnw")
                        # two Newton iterations: y = y*(1.5 - 0.5*v*y*y)
                        nc.vector.tensor_tensor(out=tnw[:], in0=vepst[:], in1=y0, op=ALU.mult)
                        nc.vector.tensor_tensor(out=tnw[:], in0=tnw[:], in1=y0, op=ALU.mult)
                        nc.vector.tensor_scalar(out=tnw[:], in0=tnw[:],
                                                scalar1=-0.5, scalar2=1.5, op0=ALU.mult, op1=ALU.add)
                        nc.vector.tensor_tensor(out=rstd[:], in0=y0, in1=tnw[:], op=ALU.mult)
                        nc.vector.tensor_tensor(out=tnw[:], in0=vepst[:], in1=rstd[:], op=ALU.mult)
                        nc.vector.tensor_tensor(out=tnw[:], in0=tnw[:], in1=rstd[:], op=ALU.mult)
                        nc.vector.tensor_scalar(out=tnw[:], in0=tnw[:],
                                                scalar1=-0.5, scalar2=1.5, op0=ALU.mult, op1=ALU.add)
                        nc.vector.tensor_tensor(out=rstd[:], in0=rstd[:], in1=tnw[:], op=ALU.mult)

                        for j, t in enumerate(ts):
                            zps = zpss[j]
                            xh = work.tile([128, 256], dt.bfloat16, tag="xh")
                            nc.vector.tensor_scalar(out=xh[:], in0=zps[:],
                                                    scalar1=negmvt[:, 2 * j:2 * j + 1],
                                                    scalar2=rstd[:, 2 * j + 1:2 * j + 2],
                                                    op0=ALU.add, op1=ALU.mult)
                            gbg, gi = gb_tiles[t]
                            y = work.tile([128, 256], dt.bfloat16, tag="y")
                            nc.vector.tensor_tensor(out=y[:], in0=xh[:], in1=gbg[:, gi, 0:256], op=ALU.mult)
                            nc.vector.tensor_tensor(out=y[:], in0=y[:], in1=gbg[:, gi, 256:512], op=ALU.add)
                            h2 = work.tile([128, 256], dt.float32, tag="h2")
                            nc.scalar.activation(h2[:], y[:], AF.Silu)
                            nc.vector.tensor_tensor(out=res[:, t, :], in0=h2[:],
                                                    in1=res[:, t, :], op=ALU.add)
                            if not last:
                                ig = t - (t // GRP) * GRP
                                if ig == 0:
                                    ag = agstp.tile([128, GRP, 256], dt.bfloat16, tag="ag")
                                nc.scalar.activation(ag[:, ig, :], res[:, t, :], AF.Copy)
                                if t == NT - 1 or ig == GRP - 1:
                                    t0g = (t // GRP) * GRP
                                    ntl = t - t0g + 1
                                    nc.sync.dma_start(
                                        agin[(l + 1) % 2].rearrange(
                                            "(t p) f -> p t f", p=128)[:, t0g:t0g + ntl, :],
                                        ag[:, 0:ntl, :])
                                for c in range(4):
                                    if CHUNK_T[c + 1] == t + 1:
                                        ag_chunk(nc, agin[(l + 1) % 2], tabs[(l + 1) % 2], c)

                # final output: residual SBUF -> DRAM
                nc.sync.dma_start(
                    h6_out.rearrange("(t p) f -> p t f", p=128), res[:, :, :])

    nc.compile()
    return nc


def make_inputs(inp, st):
    """Build per-core in_maps."""
    n_index = np.asarray(inp["n_index"]).astype(np.int64)
    ntot = st["ntot"]

    emb = sincos_emb(inp["t"])
    embT = np.ascontiguousarray(emb.T).astype(bf16)

    film_tab = prep_weights(inp)
    film_dev = film_tab.reshape(6 * 257, 512).astype(bf16)
    wconv = np.asarray(inp["conv2_w"], np.float32).reshape(6 * 256, 256).astype(bf16)
    bconv = np.asarray(inp["conv2_b"], np.float32).astype(bf16)

    w_in32 = np.zeros((32, 256), bf16)
    w_in32[:NF] = np.asarray(inp["in_conv_w"], np.float32).astype(bf16)
    we32 = np.zeros((32, 256), bf16)
    we32[:EF] = np.asarray(inp["edge_w"], np.float32).astype(bf16)
    we32[EF] = np.asarray(inp["edge_b"], np.float32).astype(bf16)
    inb = np.asarray(inp["in_conv_b"], np.float32).astype(bf16)[None, :]

    tb1 = np.ascontiguousarray(np.asarray(inp["t_b1"], np.float32).reshape(2, 128).T)
    tb2 = np.ascontiguousarray(np.asarray(inp["t_b2"], np.float32).reshape(2, 128).T)
    w1 = np.asarray(inp["t_w1"], np.float32).astype(bf16)
    w2 = np.asarray(inp["t_w2"], np.float32).astype(bf16)

    axes = host_input_aggregates(inp, st)
    S = build_S(st)

    in_maps = []
    for k in range(NCORES):
        S_dev = np.ascontiguousarray(S[k].transpose(1, 0, 2)).reshape(128, ntot * 128)
        gidx_dev = wrap_idx(st["gidx"][k])
        gvals = np.zeros(TR, np.int16)
        gvals[:RPC] = n_index[k * RPC:(k + 1) * RPC].astype(np.int16)
        nidx_dev = wrap_idx(gvals.reshape(NT, 128))
        in_maps.append({
            "S_in": S_dev, "gidx": gidx_dev,
            "nidx": nidx_dev, "embT": embT, "w1": w1, "w2": w2,
            "tb1": tb1, "tb2": tb2, "film": film_dev, "wconv": wconv,
            "bconv": bconv, "win_w": w_in32, "we_w": we32, "inb": inb,
            "axe": axes[k],
        })
    return in_maps


def run(inp, trace=False, nlayers=6):
    src = np.asarray(inp["src"]).astype(np.int64)
    dst = np.asarray(inp["dst"]).astype(np.int64)
    st = build_edge_structure(src, dst)
    nc = build(st, nlayers=nlayers)
    in_maps = make_inputs(inp, st)
    res = bass_utils.run_bass_kernel_spmd(
        nc, in_maps, core_ids=list(range(NCORES)), trace=trace,
        trace_cores=[0] if trace else None)
    h6 = np.concatenate([res.results[k]["h6_out"][:RPC] for k in range(NCORES)], 0)
    out = host_pool_head(h6, np.asarray(inp["n_index"]), inp["head_w"], inp["head_b"])
    return out, res, h6


# revision 29
# speedup vs baseline: 3.7220x; 1.9202x over previous
"""Self-contained Trainium2 Bass kernel for nn_GNN_75436805587134.

kernel(**inputs) -> np.ndarray [1024, 1]

Strategy: dst-sharded message passing across 8 NeuronCores; bf16-replicated
node-state table updated via CHUNKED AllGathers overlapped with compute;
4-bank dma_gather for h[src] rows; edge aggregation as one-hot S-tile
matmuls built ON-CHIP (iota + is_equal); LayerNorm via bn_stats + batched
Newton rsqrt (no activation-table thrash); residual kept fp32 in SBUF;
input featurization + graph pooling + head on host.
"""
import sys
for _p in ("/opt/trn_rl_repo",):
    if _p not in sys.path:
        sys.path.insert(0, _p)
import numpy as np
import ml_dtypes

import concourse.bass as bass
import concourse.bacc as bacc
import concourse.tile as tile
import concourse.mybir as mybir
import concourse.bass_utils as bass_utils

bf16 = ml_dtypes.bfloat16


N, E, B = 100000, 400000, 1024
NF, EF, H, C, D = 32, 16, 256, 256, 6
LN_EPS = 1e-5
NC = 8
RPC = N // NC                # 12500 real rows per core
NT = 98                      # dst tiles per core (98*128 = 12544)
TR = NT * 128                # 12544 padded rows per core
TROWS = NC * TR              # 100352 table rows
BANKS = 4
BANK = TROWS // BANKS        # 25088
WLEN = 4                     # dst-tiles per window
NW = (NT + WLEN - 1) // WLEN  # 25 windows
MAX_TILES_PER_CALL = 16
GRP = 8                      # tiles per DMA flush group
NGRP = (NT + GRP - 1) // GRP  # 13
CHUNK_T = [0, 24, 48, 72, 98]  # AllGather chunk boundaries (tile idx)


CHUNK_T_ARR = np.array(CHUNK_T)


def trow_of(v):
    """global node id -> padded table row (chunk-major layout).

    Table layout: [chunk c][core k][rows of chunk c], so that each chunked
    AllGather (in: agin rows of chunk c; out: concat over cores) writes one
    contiguous region.
    """
    v = np.asarray(v)
    k = v // RPC
    r = v - k * RPC                      # padded local row, < TR
    t = r // 128
    c = np.searchsorted(CHUNK_T_ARR, t, side="right") - 1
    base = NC * 128 * CHUNK_T_ARR[c]
    rows_c = (CHUNK_T_ARR[c + 1] - CHUNK_T_ARR[c]) * 128
    return base + k * rows_c + (r - CHUNK_T_ARR[c] * 128)


def build_edge_structure(src, dst):
    """Window-packed SPMD structure: edges grouped per (window, bank) and
    packed into 128-row gather tiles with NO per-dst-tile splitting. Each
    tile's one-hot S block spans the seg range its edges touch, so one wide
    matmul per tile (per feature half) accumulates into the window PSUM.

    Uniform across cores: tiles_per (w, b) = max over cores.
    Returns tile metadata + per-core gather/S data.
    """
    src = np.asarray(src).astype(np.int64)
    dst = np.asarray(dst).astype(np.int64)
    deg_out = np.maximum(np.bincount(src, minlength=N), 1.0)
    deg_in = np.maximum(np.bincount(dst, minlength=N), 1.0)
    no = deg_out ** -0.5
    ni = deg_in ** -0.5
    w_edge = (no[src] * ni[dst]).astype(np.float32)
    ratio = (1.0 / deg_in[dst] / w_edge).astype(np.float32)  # w2/w per edge

    trow = trow_of(src)
    bank = trow // BANK
    brow = trow % BANK

    core = dst // RPC
    dloc = dst - core * RPC
    t_of_e = dloc // 128
    w_of_e = t_of_e // WLEN
    drel = dloc - t_of_e * 128

    # counts per (core, w, bank) -> tiles per (w, bank) = max over cores
    cnt = np.zeros((NC, NW, BANKS), np.int64)
    np.add.at(cnt, (core, w_of_e, bank), 1)
    ntiles_wb = np.maximum(np.ceil(cnt / 128).astype(np.int64).max(axis=0), 1)  # [NW, BANKS]

    ntot = int(ntiles_wb.sum())
    # global tile ordering: w-major, then bank, then tile-in-run
    tile_w = []
    call_list = []   # per (w, b): (bank, tile_start, n_tiles)
    for w in range(NW):
        for b in range(BANKS):
            n = int(ntiles_wb[w, b])
            call_list.append((b, len(tile_w), n))
            tile_w.extend([w] * n)
    tile_w = np.array(tile_w)

    gidx = np.zeros((NC, ntot, 128), np.int16)
    sval = np.zeros((NC, ntot, 128), np.float32)   # folded w (0 for pads)
    scol = np.zeros((NC, ntot, 128), np.int32)     # window-relative dst col
    # per-(core,tile) seg span
    seg_lo = np.full((NC, ntot), 0, np.int64)
    seg_hi = np.full((NC, ntot), 0, np.int64)

    tile_start_of = {}
    for i, (b, a, n) in enumerate(call_list):
        w = tile_w[a]
        tile_start_of[(int(w), int(b))] = a

    order = np.lexsort((drel, t_of_e, bank, w_of_e, core))
    key_core = core[order]
    key_w = w_of_e[order]
    key_b = bank[order]
    import itertools
    for (k, w, b), grp in itertools.groupby(
            range(len(order)), key=lambda i: (key_core[i], key_w[i], key_b[i])):
        grp = list(grp)
        eids = order[grp]
        base = tile_start_of[(int(w), int(b))]
        for j, e in enumerate(eids):
            ti = base + j // 128
            jj = j % 128
            gidx[k, ti, jj] = brow[e]
            sval[k, ti, jj] = w_edge[e]
            seg = t_of_e[e] - w * WLEN
            scol[k, ti, jj] = seg * 128 + drel[e]
            if jj == 0:
                seg_lo[k, ti] = seg
            seg_hi[k, ti] = seg

    # uniform S-block span per tile = union over cores (edges sorted by seg)
    tspan_lo = seg_lo.min(axis=0)
    tspan_hi = seg_hi.max(axis=0)
    # pad rows (sval 0) keep scol 0 which may fall outside the span; clamp
    # them into the span start (harmless: weight 0).
    # S column offsets within the layer's S_in stream
    s_off = np.zeros(ntot, np.int64)
    acc = 0
    for ti in range(ntot):
        s_off[ti] = acc
        acc += int(tspan_hi[ti] - tspan_lo[ti] + 1) * 128
    s_cols = int(acc)

    return dict(call_list=call_list, ntot=ntot, tile_w=tile_w,
                tspan_lo=tspan_lo, tspan_hi=tspan_hi, s_off=s_off, s_cols=s_cols,
                gidx=gidx, sval=sval, scol=scol,
                no=no, ni=ni, deg_in=deg_in, ratio=ratio)


def build_S(st):
    """[NC][128, s_cols] bf16 one-hot*weight, tile blocks concatenated."""
    NCn, ntot = st["sval"].shape[0], st["ntot"]
    s_cols = st["s_cols"]
    s_off = st["s_off"]
    lo = st["tspan_lo"]
    hi = st["tspan_hi"]
    out = []
    j = np.arange(128)
    for k in range(NCn):
        S = np.zeros((128, s_cols), bf16)
        for ti in range(ntot):
            width = int(hi[ti] - lo[ti] + 1) * 128
            col = st["scol"][k, ti] - int(lo[ti]) * 128
            # pads (sval 0) may fall outside the span; clamp to col 0
            col = np.where((col < 0) | (col >= width), 0, col)
            S[j, s_off[ti] + col] = st["sval"][k, ti].astype(bf16)
        out.append(S)
    return out


def wrap_idx(idx_tiles):
    """[ntot,128] int16 -> [128, ntot*8] wrapped+replicated layout"""
    ntot = idx_tiles.shape[0]
    out = np.zeros((128, ntot * 8), np.int16)
    for ti in range(ntot):
        w = idx_tiles[ti].reshape(8, 16).T  # [16, 8]
        out[:, ti * 8:(ti + 1) * 8] = np.tile(w, (8, 1))
    return out


def prep_weights(inp):
    """Fold LN gammas into film tables; build device weight arrays."""
    g1 = np.asarray(inp["ln2_g"], np.float32)    # layer path uses ln2/film2/conv2
    b1 = np.asarray(inp["ln2_b"], np.float32)
    fw = np.asarray(inp["film2_w"], np.float32)  # [D, C, 2H]
    fb = np.asarray(inp["film2_b"], np.float32)  # [D, 2H]
    film_tab = np.zeros((D, C + 1, 2 * H), np.float32)
    for i in range(D):
        gam_w = fw[i, :, :H]; bet_w = fw[i, :, H:]
        gam_b = fb[i, :H]; bet_b = fb[i, H:]
        gl = g1[i]; bl = b1[i]
        # Gamma_eff = gl*(1+gamma);  Beta_eff = bl*(1+gamma) + beta
        film_tab[i, :C, :H] = gam_w * gl[None, :]
        film_tab[i, C, :H] = gl * (1.0 + gam_b)
        film_tab[i, :C, H:] = bet_w + gam_w * bl[None, :]
        film_tab[i, C, H:] = bet_b + bl * (1.0 + gam_b)
    return film_tab


def sincos_emb(t):
    half = 64
    freqs = np.exp(-np.log(1000.0) * np.arange(half, dtype=np.float32) / half)
    a = (np.asarray(t, np.float32) * 1000.0)[:, None] * freqs[None, :]
    return np.concatenate([np.sin(a), np.cos(a)], axis=-1).astype(np.float32)  # [B,128]


def host_pool_head(h6_full_real, n_index, head_w, head_b):
    """h6_full_real: [N, H] f32 (real rows). Returns [B, 1]."""
    n_index = np.asarray(n_index).astype(np.int64)
    cnt = np.maximum(np.bincount(n_index, minlength=B), 1.0)
    pooled = np.zeros((B, H), np.float64)
    np.add.at(pooled, n_index, h6_full_real.astype(np.float64))
    g_mean = (pooled / cnt[:, None]).astype(np.float32)
    return g_mean @ np.asarray(head_w, np.float32) + np.asarray(head_b, np.float32)


def host_input_aggregates(inp, st):
    """Exact f32 input-stage aggregation per core -> [NC][64, TR] bf16 (transposed)."""
    src = np.asarray(inp["src"]).astype(np.int64)
    dst = np.asarray(inp["dst"]).astype(np.int64)
    w_e = (st["no"][src] * st["ni"][dst]).astype(np.float32)
    nx = np.asarray(inp["node_x"], np.float32)
    agg_x = np.zeros((N, NF), np.float32)
    np.add.at(agg_x, dst, w_e[:, None] * nx[src])
    ratio = st["ratio"]
    ee = np.asarray(inp["edge_e"], np.float32) * ratio[:, None]
    e_aug = np.concatenate([ee, ratio[:, None]], 1)
    agg_e = np.zeros((N, 17), np.float32)
    np.add.at(agg_e, dst, w_e[:, None] * e_aug)
    outs = []
    for k in range(NC):
        a = np.zeros((64, TR), np.float32)
        a[0:NF, :RPC] = agg_x[k * RPC:(k + 1) * RPC].T
        a[32:49, :RPC] = agg_e[k * RPC:(k + 1) * RPC].T
        outs.append(a.astype(bf16))
    return outs


def kernel(**inputs):
    out, _res, _h6 = run(inputs, trace=False)
    return out.astype(np.float32)


dt = mybir.dt
AF = mybir.ActivationFunctionType
ALU = mybir.AluOpType
NCORES = NC
MAGIC = 0x5f3759df


def build(st, nlayers=6):
    ntot = st["ntot"]
    tile_w = st["tile_w"]
    call_list = st["call_list"]
    s_off = st["s_off"]
    s_cols = st["s_cols"]
    tlo = st["tspan_lo"]
    thi = st["tspan_hi"]

    nc = bacc.Bacc("TRN2", target_bir_lowering=False, debug=False,
                   enable_asserts=False, num_devices=NCORES, num_swdge_queues=4)

    # ---------- I/O ----------
    S_in = nc.dram_tensor("S_in", [128, s_cols], dt.bfloat16, kind="ExternalInput").ap()
    gidx = nc.dram_tensor("gidx", [128, ntot * 8], dt.int16, kind="ExternalInput").ap()
    nidx = nc.dram_tensor("nidx", [128, NT * 8], dt.int16, kind="ExternalInput").ap()
    embT = nc.dram_tensor("embT", [128, B], dt.bfloat16, kind="ExternalInput").ap()
    w1 = nc.dram_tensor("w1", [128, 256], dt.bfloat16, kind="ExternalInput").ap()
    w2 = nc.dram_tensor("w2", [256, 256], dt.bfloat16, kind="ExternalInput").ap()
    tb1 = nc.dram_tensor("tb1", [128, 2], dt.float32, kind="ExternalInput").ap()
    tb2 = nc.dram_tensor("tb2", [128, 2], dt.float32, kind="ExternalInput").ap()
    film = nc.dram_tensor("film", [6 * 257, 512], dt.bfloat16, kind="ExternalInput").ap()
    wconv = nc.dram_tensor("wconv", [6 * 256, 256], dt.bfloat16, kind="ExternalInput").ap()
    bconv = nc.dram_tensor("bconv", [6, 256], dt.bfloat16, kind="ExternalInput").ap()
    win_w = nc.dram_tensor("win_w", [32, 256], dt.bfloat16, kind="ExternalInput").ap()
    we_w = nc.dram_tensor("we_w", [32, 256], dt.bfloat16, kind="ExternalInput").ap()
    inb = nc.dram_tensor("inb", [1, 256], dt.bfloat16, kind="ExternalInput").ap()
    axe = nc.dram_tensor("axe", [64, TR], dt.bfloat16, kind="ExternalInput").ap()
    h6_out = nc.dram_tensor("h6_out", [TR, 256], dt.float32, kind="ExternalOutput").ap()

    # ---------- internal DRAM ----------
    agin = [nc.dram_tensor(f"agin{i}", [TR, 256], dt.bfloat16, kind="Internal").ap()
            for i in range(2)]
    tabs = [nc.dram_tensor(f"tab{i}", [TROWS, 256], dt.bfloat16, kind="Internal",
                           addr_space="Shared").ap() for i in range(2)]
    gbd = nc.dram_tensor("gbd", [6 * B, 512], dt.bfloat16, kind="Internal").ap()

    RG = [list(range(NCORES))]

    def windows_of():
        out = []
        ci = 0
        for w in range(NW):
            ts = list(range(w * WLEN, min((w + 1) * WLEN, NT)))
            calls = []
            while ci < len(call_list):
                b, a, n = call_list[ci]
                if tile_w[a] != w:
                    break
                calls.append((b, a, n))
                ci += 1
            out.append((w, ts, calls))
        return out

    WINS = windows_of()
    W_MAX = max(sum(n for (_, _, n) in calls) for (_, _, calls) in WINS)
    SCW_MAX = 0
    for (w, ts, calls) in WINS:
        a0 = calls[0][1]
        a1 = calls[-1][1] + calls[-1][2] - 1
        w_sc = int(s_off[a1] + (thi[a1] - tlo[a1] + 1) * 128 - s_off[a0])
        SCW_MAX = max(SCW_MAX, w_sc)
    # chunk id of a tile
    def chunk_of(t):
        for c in range(4):
            if t < CHUNK_T[c + 1]:
                return c
        return 3

    def ag_chunk(nc, src, dstb, c):
        # chunk-major table: AG chunk c writes the contiguous region
        # [NC*r0, NC*r1) of the table (concat over cores of chunk-c rows).
        r0, r1 = CHUNK_T[c] * 128, CHUNK_T[c + 1] * 128
        in_ap = src[r0:r1, :]
        out_ap = dstb[NCORES * r0:NCORES * r1, :]
        nc.gpsimd.collective_compute(
            "AllGather", ALU.bypass, replica_groups=RG,
            ins=[in_ap.opt()], outs=[out_ap.opt()])

    with tile.TileContext(nc) as tc:
        with tc.tile_pool(name="const", bufs=1) as constp, \
             tc.tile_pool(name="resp", bufs=1) as resp, \
             tc.tile_pool(name="wpool", bufs=2) as wpool, \
             tc.tile_pool(name="gpool", bufs=2) as gpool, \
             tc.tile_pool(name="spool", bufs=2) as spool, \
             tc.tile_pool(name="gbpool", bufs=3) as gbpool, \
             tc.tile_pool(name="agst", bufs=2) as agstp, \
             tc.tile_pool(name="work", bufs=3) as work, \
             tc.tile_pool(name="tiny", bufs=3) as tiny, \
             tc.tile_pool(name="psZ", bufs=5, space="PSUM") as psZ:

            # ---- resident constants ----
            gidx_sb = constp.tile([128, ntot * 8], dt.int16)
            nc.sync.dma_start(gidx_sb[:], gidx[:])
            nidx_sb = constp.tile([128, NT * 8], dt.int16)
            nc.sync.dma_start(nidx_sb[:], nidx[:])
            ones1 = constp.tile([1, 128], dt.bfloat16)
            nc.vector.memset(ones1[:], 1.0)
            win_sb = constp.tile([32, 256], dt.bfloat16)
            nc.sync.dma_start(win_sb[:], win_w[:])
            we_sb = constp.tile([32, 256], dt.bfloat16)
            nc.sync.dma_start(we_sb[:], we_w[:])
            inb_sb = constp.tile([1, 256], dt.bfloat16)
            nc.sync.dma_start(inb_sb[:], inb[:])

            # residual state, fp32, resident in SBUF
            res = resp.tile([128, NT, 256], dt.float32)

            # ---- phase B: input stage (aggregates precomputed on host) ----
            with tc.tile_pool(name="axp", bufs=2) as axp:
                for g in range(NGRP):
                    t0g = g * GRP
                    ntl = min(GRP, NT - t0g)
                    ax_sb = axp.tile([32, GRP * 128], dt.bfloat16, tag="axw")
                    nc.sync.dma_start(ax_sb[:, 0:ntl * 128],
                                      axe[0:32, t0g * 128:(t0g + ntl) * 128])
                    ae_sb = axp.tile([32, GRP * 128], dt.bfloat16, tag="aew")
                    nc.sync.dma_start(ae_sb[:, 0:ntl * 128],
                                      axe[32:64, t0g * 128:(t0g + ntl) * 128])
                    ag = agstp.tile([128, GRP, 256], dt.bfloat16, tag="ag")
                    for i in range(ntl):
                        t = t0g + i
                        zps = psZ.tile([128, 256], dt.float32, tag="z", space="PSUM")
                        nc.tensor.matmul(zps[:], ax_sb[:, i * 128:(i + 1) * 128],
                                         win_sb[:32, :], start=True, stop=False)
                        nc.tensor.matmul(zps[:], ae_sb[:, i * 128:(i + 1) * 128],
                                         we_sb[:32, :], start=False, stop=False)
                        nc.tensor.matmul(zps[:], ones1[:], inb_sb[:], start=False, stop=True)
                        nc.scalar.activation(res[:, t, :], zps[:], AF.Copy)
                        nc.vector.tensor_copy(ag[:, i, :], zps[:])
                    nc.sync.dma_start(
                        agin[0].rearrange("(t p) f -> p t f", p=128)[:, t0g:t0g + ntl, :],
                        ag[:, 0:ntl, :])
                    # fire AG chunks as their groups complete
                    for c in range(4):
                        if CHUNK_T[c + 1] == t0g + ntl:
                            ag_chunk(nc, agin[0], tabs[0], c)

            # ---- phase A: cond -> gamma/beta tables (overlaps phase B DMA) ----
            with tc.tile_pool(name="condp", bufs=1) as condp, \
                 tc.tile_pool(name="psC", bufs=2, space="PSUM") as psC:
                embT_sb = condp.tile([128, B], dt.bfloat16)
                nc.sync.dma_start(embT_sb[:], embT[:])
                w1_sb = condp.tile([128, 256], dt.bfloat16)
                nc.sync.dma_start(w1_sb[:], w1[:])
                w2_sb = condp.tile([128, 2, 256], dt.bfloat16)
                nc.sync.dma_start(w2_sb[:], w2[:].rearrange("(a p) n -> p a n", p=128))
                tb1_sb = condp.tile([128, 2], dt.float32)
                nc.sync.dma_start(tb1_sb[:], tb1[:])
                tb2_sb = condp.tile([128, 2], dt.float32)
                nc.sync.dma_start(tb2_sb[:], tb2[:])

                c1T = condp.tile([128, 2, B], dt.bfloat16)  # [part, oc, t]
                for oc in range(2):
                    for tb in range(2):
                        ps = psC.tile([128, 512], dt.float32, tag="condps", space="PSUM")
                        nc.tensor.matmul(ps[:], w1_sb[:, oc * 128:(oc + 1) * 128],
                                         embT_sb[:, tb * 512:(tb + 1) * 512],
                                         start=True, stop=True)
                        nc.scalar.activation(c1T[:, oc, tb * 512:(tb + 1) * 512], ps[:],
                                             AF.Silu, bias=tb1_sb[:, oc:oc + 1])
                c2T = condp.tile([128, 2, B], dt.bfloat16)
                for oc in range(2):
                    for tb in range(2):
                        ps = psC.tile([128, 512], dt.float32, tag="condps", space="PSUM")
                        for k in range(2):
                            nc.tensor.matmul(ps[:], w2_sb[:, k, oc * 128:(oc + 1) * 128],
                                             c1T[:, k, tb * 512:(tb + 1) * 512],
                                             start=(k == 0), stop=(k == 1))
                        nc.scalar.activation(c2T[:, oc, tb * 512:(tb + 1) * 512], ps[:],
                                             AF.Identity, bias=tb2_sb[:, oc:oc + 1])
                for l in range(6):
                    film_sb = wpool.tile([128, 2, 512], dt.bfloat16, tag="film")
                    nc.sync.dma_start(film_sb[:], film[l * 257:l * 257 + 256, :]
                                      .rearrange("(a p) n -> p a n", p=128))
                    filmb_sb = wpool.tile([1, 512], dt.bfloat16, tag="filmb")
                    nc.sync.dma_start(filmb_sb[:], film[l * 257 + 256:l * 257 + 257, :])
                    for gc in range(8):
                        ps = psC.tile([128, 512], dt.float32, tag="condps", space="PSUM")
                        for k in range(2):
                            nc.tensor.matmul(ps[:], c2T[:, k, gc * 128:(gc + 1) * 128],
                                             film_sb[:, k, :], start=(k == 0), stop=False)
                        nc.tensor.matmul(ps[:], ones1[:], filmb_sb[:],
                                         start=False, stop=True)
                        gbt = tiny.tile([128, 512], dt.bfloat16, tag="gbt")
                        nc.vector.tensor_copy(gbt[:], ps[:])
                        nc.sync.dma_start(
                            gbd[(l * B + gc * 128):(l * B + (gc + 1) * 128), :], gbt[:])

            # ---- phase C: layers ----
            with tc.tile_pool(name="psAgg", bufs=1, space="PSUM") as psAgg:
                for l in range(nlayers):
                    table = tabs[l % 2]
                    last = l == nlayers - 1

                    wl_sb = wpool.tile([128, 2, 256], dt.bfloat16, tag="wl")
                    nc.sync.dma_start(wl_sb[:], wconv[l * 256:(l + 1) * 256, :]
                                      .rearrange("(a p) n -> p a n", p=128))
                    bl_sb = wpool.tile([1, 256], dt.bfloat16, tag="bl")
                    nc.sync.dma_start(bl_sb[:], bconv[l:l + 1, :])

                    gb_tiles = {}

                    def gb_fetch(wi):
                        if wi >= NW:
                            return
                        t0c = wi * WLEN
                        ntl = min(WLEN, NT - t0c)
                        gbg = gbpool.tile([128, 4, 512], dt.bfloat16, tag="gbg")
                        nc.gpsimd.dma_gather(
                            gbg[:, 0:ntl, :], gbd[l * B:(l + 1) * B, :],
                            nidx_sb[:, t0c * 8:(t0c + ntl) * 8], ntl * 128, ntl * 128, 512,
                            queue_num=wi % 4)
                        for i in range(ntl):
                            gb_tiles[t0c + i] = (gbg, i)

                    gb_fetch(0)
                    gb_fetch(1)
                    qi = 0
                    ag = None
                    for (w, ts, calls) in WINS:
                        alo = psAgg.tile([128, 512], dt.float32, tag="alo", space="PSUM")
                        ahi = psAgg.tile([128, 512], dt.float32, tag="ahi", space="PSUM")
                        w_first = calls[0][1]
                        w_last = calls[-1][1] + calls[-1][2] - 1
                        ws0 = int(s_off[w_first])
                        w_sc = int(s_off[w_last] + (thi[w_last] - tlo[w_last] + 1) * 128) - ws0
                        s_sb = spool.tile([128, SCW_MAX], dt.bfloat16, tag="s")
                        nc.sync.dma_start(s_sb[:, 0:w_sc], S_in[:, ws0:ws0 + w_sc])
                        g_win = gpool.tile([128, W_MAX, 256], dt.bfloat16, tag="g")
                        off = 0
                        for (b, a, n) in calls:
                            nc.gpsimd.dma_gather(
                                g_win[:, off:off + n, :], table[b * BANK:(b + 1) * BANK, :],
                                gidx_sb[:, a * 8:(a + n) * 8], n * 128, n * 128, 256,
                                queue_num=qi % 4)
                            qi += 1
                            for i in range(n):
                                ti = a + i
                                lo = int(tlo[ti])
                                width = (int(thi[ti]) - lo + 1) * 128
                                sc0 = int(s_off[ti]) - ws0
                                st_f = ti == w_first
                                sp_f = ti == w_last
                                nc.tensor.matmul(alo[:, lo * 128:lo * 128 + width],
                                                 g_win[:, off + i, 0:128],
                                                 s_sb[:, sc0:sc0 + width],
                                                 start=st_f, stop=sp_f)
                                nc.tensor.matmul(ahi[:, lo * 128:lo * 128 + width],
                                                 g_win[:, off + i, 128:256],
                                                 s_sb[:, sc0:sc0 + width],
                                                 start=st_f, stop=sp_f)
                            off += n
                        gb_fetch(w + 2)
                        # post-process the window's 4 dst tiles
                        zpss = []
                        st6 = tiny.tile([128, WLEN * 6], dt.float32, tag="st6")
                        mvt = tiny.tile([128, WLEN * 2], dt.float32, tag="mvt")
                        for j, t in enumerate(ts):
                            seg = t % WLEN
                            ab_lo = work.tile([128, 128], dt.bfloat16, tag="ablo")
                            nc.scalar.activation(ab_lo[:], alo[:, seg * 128:(seg + 1) * 128],
                                                 AF.Copy)
                            ab_hi = work.tile([128, 128], dt.bfloat16, tag="abhi")
                            nc.scalar.activation(ab_hi[:], ahi[:, seg * 128:(seg + 1) * 128],
                                                 AF.Copy)
                            zps = psZ.tile([128, 256], dt.float32, tag="z", space="PSUM")
                            nc.tensor.matmul(zps[:], ab_lo[:], wl_sb[:, 0, :], start=True, stop=False)
                            nc.tensor.matmul(zps[:], ab_hi[:], wl_sb[:, 1, :], start=False, stop=False)
                            nc.tensor.matmul(zps[:], ones1[:], bl_sb[:], start=False, stop=True)
                            zpss.append(zps)
                            nc.vector.bn_stats(st6[:, j * 6:(j + 1) * 6], zps[:])
                            nc.vector.bn_aggr(mvt[:, j * 2:(j + 1) * 2], st6[:, j * 6:(j + 1) * 6])
                        # full-tile LN scale: rsqrt(var+eps) via bit-trick + 2 Newton
                        # iterations run on the interleaved (mean,var) tile; the
                        # rsqrt of the mean columns is computed but unused.
                        vepst = tiny.tile([128, WLEN * 2], dt.float32, tag="vepst")
                        nc.vector.tensor_scalar_add(vepst[:], mvt[:], LN_EPS)
                        negmvt = tiny.tile([128, WLEN * 2], dt.float32, tag="negmvt")
                        nc.vector.tensor_scalar_mul(negmvt[:], mvt[:], -1.0)
                        yi = tiny.tile([128, WLEN * 2], dt.int32, tag="yi")
                        nc.vector.tensor_scalar(out=yi[:], in0=vepst[:].bitcast(dt.int32),
                                                scalar1=1, scalar2=None,
                                                op0=ALU.arith_shift_right)
                        nc.vector.tensor_scalar(out=yi[:], in0=yi[:],
                                                scalar1=-1, scalar2=MAGIC,
                                                op0=ALU.mult, op1=ALU.add)
                        y0 = yi[:].bitcast(dt.float32)
                        rstd = tiny.tile([128, WLEN * 2], dt.float32, tag="rstd")
                        tnw = tiny.tile([128, WLEN * 2], dt.float32, tag="tnw")
                        # two Newton iterations: y = y*(1.5 - 0.5*v*y*y)
                        nc.vector.tensor_tensor(out=tnw[:], in0=vepst[:], in1=y0, op=ALU.mult)
                        nc.vector.tensor_tensor(out=tnw[:], in0=tnw[:], in1=y0, op=ALU.mult)
                        nc.vector.tensor_scalar(out=tnw[:], in0=tnw[:],
                                                scalar1=-0.5, scalar2=1.5, op0=ALU.mult, op1=ALU.add)
                        nc.vector.tensor_tensor(out=rstd[:], in0=y0, in1=tnw[:], op=ALU.mult)
                        nc.vector.tensor_tensor(out=tnw[:], in0=vepst[:], in1=rstd[:], op=ALU.mult)
                        nc.vector.tensor_tensor(out=tnw[:], in0=tnw[:], in1=rstd[:], op=ALU.mult)
                        nc.vector.tensor_scalar(out=tnw[:], in0=tnw[:],
                                                scalar1=-0.5, scalar2=1.5, op0=ALU.mult, op1=ALU.add)
                        nc.vector.tensor_tensor(out=rstd[:], in0=rstd[:], in1=tnw[:], op=ALU.mult)

                        for j, t in enumerate(ts):
                            zps = zpss[j]
                            xh = work.tile([128, 256], dt.bfloat16, tag="xh")
                            nc.vector.tensor_scalar(out=xh[:], in0=zps[:],
                                                    scalar1=negmvt[:, 2 * j:2 * j + 1],
                                                    scalar2=rstd[:, 2 * j + 1:2 * j + 2],
                                                    op0=ALU.add, op1=ALU.mult)
                            gbg, gi = gb_tiles[t]
                            y = work.tile([128, 256], dt.bfloat16, tag="y")
                            nc.vector.tensor_tensor(out=y[:], in0=xh[:], in1=gbg[:, gi, 0:256], op=ALU.mult)
                            nc.vector.tensor_tensor(out=y[:], in0=y[:], in1=gbg[:, gi, 256:512], op=ALU.add)
                            h2 = work.tile([128, 256], dt.bfloat16, tag="h2")
                            nc.scalar.activation(h2[:], y[:], AF.Silu)
                            nc.vector.tensor_tensor(out=res[:, t, :], in0=h2[:],
                                                    in1=res[:, t, :], op=ALU.add)
                            if not last:
                                ig = t - (t // GRP) * GRP
                                if ig == 0:
                                    ag = agstp.tile([128, GRP, 256], dt.bfloat16, tag="ag")
                                nc.scalar.activation(ag[:, ig, :], res[:, t, :], AF.Copy)
                                if t == NT - 1 or ig == GRP - 1:
                                    t0g = (t // GRP) * GRP
                                    ntl = t - t0g + 1
                                    nc.sync.dma_start(
                                        agin[(l + 1) % 2].rearrange(
                                            "(t p) f -> p t f", p=128)[:, t0g:t0g + ntl, :],
                                        ag[:, 0:ntl, :])
                                for c in range(4):
                                    if CHUNK_T[c + 1] == t + 1:
                                        ag_chunk(nc, agin[(l + 1) % 2], tabs[(l + 1) % 2], c)

                # final output: residual SBUF -> DRAM
                nc.sync.dma_start(
                    h6_out.rearrange("(t p) f -> p t f", p=128), res[:, :, :])

    nc.compile()
    return nc


def make_inputs(inp, st):
    """Build per-core in_maps."""
    n_index = np.asarray(inp["n_index"]).astype(np.int64)
    ntot = st["ntot"]

    emb = sincos_emb(inp["t"])
    embT = np.ascontiguousarray(emb.T).astype(bf16)

    film_tab = prep_weights(inp)
    film_dev = film_tab.reshape(6 * 257, 512).astype(bf16)
    wconv = np.asarray(inp["conv2_w"], np.float32).reshape(6 * 256, 256).astype(bf16)
    bconv = np.asarray(inp["conv2_b"], np.float32).astype(bf16)

    w_in32 = np.zeros((32, 256), bf16)
    w_in32[:NF] = np.asarray(inp["in_conv_w"], np.float32).astype(bf16)
    we32 = np.zeros((32, 256), bf16)
    we32[:EF] = np.asarray(inp["edge_w"], np.float32).astype(bf16)
    we32[EF] = np.asarray(inp["edge_b"], np.float32).astype(bf16)
    inb = np.asarray(inp["in_conv_b"], np.float32).astype(bf16)[None, :]

    tb1 = np.ascontiguousarray(np.asarray(inp["t_b1"], np.float32).reshape(2, 128).T)
    tb2 = np.ascontiguousarray(np.asarray(inp["t_b2"], np.float32).reshape(2, 128).T)
    w1 = np.asarray(inp["t_w1"], np.float32).astype(bf16)
    w2 = np.asarray(inp["t_w2"], np.float32).astype(bf16)

    axes = host_input_aggregates(inp, st)
    S = build_S(st)

    in_maps = []
    for k in range(NCORES):
        S_dev = S[k]
        gidx_dev = wrap_idx(st["gidx"][k])
        gvals = np.zeros(TR, np.int16)
        gvals[:RPC] = n_index[k * RPC:(k + 1) * RPC].astype(np.int16)
        nidx_dev = wrap_idx(gvals.reshape(NT, 128))
        in_maps.append({
            "S_in": S_dev, "gidx": gidx_dev,
            "nidx": nidx_dev, "embT": embT, "w1": w1, "w2": w2,
            "tb1": tb1, "tb2": tb2, "film": film_dev, "wconv": wconv,
            "bconv": bconv, "win_w": w_in32, "we_w": we32, "inb": inb,
            "axe": axes[k],
        })
    return in_maps


def run(inp, trace=False, nlayers=6):
    src = np.asarray(inp["src"]).astype(np.int64)
    dst = np.asarray(inp["dst"]).astype(np.int64)
    st = build_edge_structure(src, dst)
    nc = build(st, nlayers=nlayers)
    in_maps = make_inputs(inp, st)
    res = bass_utils.run_bass_kernel_spmd(
        nc, in_maps, core_ids=list(range(NCORES)), trace=trace,
        trace_cores=[0] if trace else None)
    h6 = np.concatenate([res.results[k]["h6_out"][:RPC] for k in range(NCORES)], 0)
    out = host_pool_head(h6, np.asarray(inp["n_index"]), inp["head_w"], inp["head_b"])
    return out, res, h6


# revision 30
# speedup vs baseline: 4.0316x; 1.0832x over previous
"""Self-contained Trainium2 Bass kernel for nn_GNN_75436805587134.

kernel(**inputs) -> np.ndarray [1024, 1]

Strategy: dst-sharded message passing across 8 NeuronCores; bf16-replicated
node-state table updated via CHUNKED AllGathers overlapped with compute;
4-bank dma_gather for h[src] rows; edge aggregation as one-hot S-tile
matmuls built ON-CHIP (iota + is_equal); LayerNorm via bn_stats + batched
Newton rsqrt (no activation-table thrash); residual kept fp32 in SBUF;
input featurization + graph pooling + head on host.
"""
import sys
for _p in ("/opt/trn_rl_repo",):
    if _p not in sys.path:
        sys.path.insert(0, _p)
import numpy as np
import ml_dtypes

import concourse.bass as bass
import concourse.bacc as bacc
import concourse.tile as tile
import concourse.mybir as mybir
import concourse.bass_utils as bass_utils

bf16 = ml_dtypes.bfloat16


N, E, B = 100000, 400000, 1024
NF, EF, H, C, D = 32, 16, 256, 256, 6
LN_EPS = 1e-5
NC = 8
RPC = N // NC                # 12500 real rows per core
NT = 98                      # dst tiles per core (98*128 = 12544)
TR = NT * 128                # 12544 padded rows per core
TROWS = NC * TR              # 100352 table rows
BANKS = 4
BANK = TROWS // BANKS        # 25088
WLEN = 4                     # dst-tiles per window
NW = (NT + WLEN - 1) // WLEN  # 25 windows
MAX_TILES_PER_CALL = 16
GRP = 8                      # tiles per DMA flush group
NGRP = (NT + GRP - 1) // GRP  # 13
CHUNK_T = [0, 24, 48, 72, 98]  # AllGather chunk boundaries (tile idx)


CHUNK_T_ARR = np.array(CHUNK_T)


def trow_of(v):
    """global node id -> padded table row (chunk-major layout).

    Table layout: [chunk c][core k][rows of chunk c], so that each chunked
    AllGather (in: agin rows of chunk c; out: concat over cores) writes one
    contiguous region.
    """
    v = np.asarray(v)
    k = v // RPC
    r = v - k * RPC                      # padded local row, < TR
    t = r // 128
    c = np.searchsorted(CHUNK_T_ARR, t, side="right") - 1
    base = NC * 128 * CHUNK_T_ARR[c]
    rows_c = (CHUNK_T_ARR[c + 1] - CHUNK_T_ARR[c]) * 128
    return base + k * rows_c + (r - CHUNK_T_ARR[c] * 128)


def build_edge_structure(src, dst):
    """Window-packed SPMD structure: edges grouped per (window, bank) and
    packed into 128-row gather tiles with NO per-dst-tile splitting. Each
    tile's one-hot S block spans the seg range its edges touch, so one wide
    matmul per tile (per feature half) accumulates into the window PSUM.

    Uniform across cores: tiles_per (w, b) = max over cores.
    Returns tile metadata + per-core gather/S data.
    """
    src = np.asarray(src).astype(np.int64)
    dst = np.asarray(dst).astype(np.int64)
    deg_out = np.maximum(np.bincount(src, minlength=N), 1.0)
    deg_in = np.maximum(np.bincount(dst, minlength=N), 1.0)
    no = deg_out ** -0.5
    ni = deg_in ** -0.5
    w_edge = (no[src] * ni[dst]).astype(np.float32)
    ratio = (1.0 / deg_in[dst] / w_edge).astype(np.float32)  # w2/w per edge

    trow = trow_of(src)
    bank = trow // BANK
    brow = trow % BANK

    core = dst // RPC
    dloc = dst - core * RPC
    t_of_e = dloc // 128
    w_of_e = t_of_e // WLEN
    drel = dloc - t_of_e * 128

    # counts per (core, w, bank) -> tiles per (w, bank) = max over cores
    cnt = np.zeros((NC, NW, BANKS), np.int64)
    np.add.at(cnt, (core, w_of_e, bank), 1)
    ntiles_wb = np.maximum(np.ceil(cnt / 128).astype(np.int64).max(axis=0), 1)  # [NW, BANKS]

    ntot = int(ntiles_wb.sum())
    # global tile ordering: w-major, then bank, then tile-in-run
    tile_w = []
    call_list = []   # per (w, b): (bank, tile_start, n_tiles)
    for w in range(NW):
        for b in range(BANKS):
            n = int(ntiles_wb[w, b])
            call_list.append((b, len(tile_w), n))
            tile_w.extend([w] * n)
    tile_w = np.array(tile_w)

    gidx = np.zeros((NC, ntot, 128), np.int16)
    sval = np.zeros((NC, ntot, 128), np.float32)   # folded w (0 for pads)
    scol = np.zeros((NC, ntot, 128), np.int32)     # window-relative dst col
    # per-(core,tile) seg span
    seg_lo = np.full((NC, ntot), 0, np.int64)
    seg_hi = np.full((NC, ntot), 0, np.int64)

    tile_start_of = {}
    for i, (b, a, n) in enumerate(call_list):
        w = tile_w[a]
        tile_start_of[(int(w), int(b))] = a

    order = np.lexsort((drel, t_of_e, bank, w_of_e, core))
    key_core = core[order]
    key_w = w_of_e[order]
    key_b = bank[order]
    import itertools
    for (k, w, b), grp in itertools.groupby(
            range(len(order)), key=lambda i: (key_core[i], key_w[i], key_b[i])):
        grp = list(grp)
        eids = order[grp]
        base = tile_start_of[(int(w), int(b))]
        for j, e in enumerate(eids):
            ti = base + j // 128
            jj = j % 128
            gidx[k, ti, jj] = brow[e]
            sval[k, ti, jj] = w_edge[e]
            seg = t_of_e[e] - w * WLEN
            scol[k, ti, jj] = seg * 128 + drel[e]
            if jj == 0:
                seg_lo[k, ti] = seg
            seg_hi[k, ti] = seg

    # uniform S-block span per tile = union over cores (edges sorted by seg)
    tspan_lo = seg_lo.min(axis=0)
    tspan_hi = seg_hi.max(axis=0)
    # pad rows (sval 0) keep scol 0 which may fall outside the span; clamp
    # them into the span start (harmless: weight 0).
    # S column offsets within the layer's S_in stream
    s_off = np.zeros(ntot, np.int64)
    acc = 0
    for ti in range(ntot):
        s_off[ti] = acc
        acc += int(tspan_hi[ti] - tspan_lo[ti] + 1) * 128
    s_cols = int(acc)

    return dict(call_list=call_list, ntot=ntot, tile_w=tile_w,
                tspan_lo=tspan_lo, tspan_hi=tspan_hi, s_off=s_off, s_cols=s_cols,
                gidx=gidx, sval=sval, scol=scol,
                no=no, ni=ni, deg_in=deg_in, ratio=ratio)


def build_S(st):
    """[NC][128, s_cols] fp8e4m3 one-hot*weight, tile blocks concatenated."""
    NCn, ntot = st["sval"].shape[0], st["ntot"]
    s_cols = st["s_cols"]
    s_off = st["s_off"]
    lo = st["tspan_lo"]
    hi = st["tspan_hi"]
    out = []
    j = np.arange(128)
    for k in range(NCn):
        S = np.zeros((128, s_cols), ml_dtypes.float8_e4m3)
        for ti in range(ntot):
            width = int(hi[ti] - lo[ti] + 1) * 128
            col = st["scol"][k, ti] - int(lo[ti]) * 128
            # pads (sval 0) may fall outside the span; clamp to col 0
            col = np.where((col < 0) | (col >= width), 0, col)
            S[j, s_off[ti] + col] = st["sval"][k, ti].astype(ml_dtypes.float8_e4m3)
        out.append(S)
    return out


def wrap_idx(idx_tiles):
    """[ntot,128] int16 -> [128, ntot*8] wrapped+replicated layout"""
    ntot = idx_tiles.shape[0]
    out = np.zeros((128, ntot * 8), np.int16)
    for ti in range(ntot):
        w = idx_tiles[ti].reshape(8, 16).T  # [16, 8]
        out[:, ti * 8:(ti + 1) * 8] = np.tile(w, (8, 1))
    return out


def prep_weights(inp):
    """Fold LN gammas into film tables; build device weight arrays."""
    g1 = np.asarray(inp["ln2_g"], np.float32)    # layer path uses ln2/film2/conv2
    b1 = np.asarray(inp["ln2_b"], np.float32)
    fw = np.asarray(inp["film2_w"], np.float32)  # [D, C, 2H]
    fb = np.asarray(inp["film2_b"], np.float32)  # [D, 2H]
    film_tab = np.zeros((D, C + 1, 2 * H), np.float32)
    for i in range(D):
        gam_w = fw[i, :, :H]; bet_w = fw[i, :, H:]
        gam_b = fb[i, :H]; bet_b = fb[i, H:]
        gl = g1[i]; bl = b1[i]
        # Gamma_eff = gl*(1+gamma);  Beta_eff = bl*(1+gamma) + beta
        film_tab[i, :C, :H] = gam_w * gl[None, :]
        film_tab[i, C, :H] = gl * (1.0 + gam_b)
        film_tab[i, :C, H:] = bet_w + gam_w * bl[None, :]
        film_tab[i, C, H:] = bet_b + bl * (1.0 + gam_b)
    return film_tab


def sincos_emb(t):
    half = 64
    freqs = np.exp(-np.log(1000.0) * np.arange(half, dtype=np.float32) / half)
    a = (np.asarray(t, np.float32) * 1000.0)[:, None] * freqs[None, :]
    return np.concatenate([np.sin(a), np.cos(a)], axis=-1).astype(np.float32)  # [B,128]


def host_pool_head(h6_full_real, n_index, head_w, head_b):
    """h6_full_real: [N, H] f32 (real rows). Returns [B, 1]."""
    n_index = np.asarray(n_index).astype(np.int64)
    cnt = np.maximum(np.bincount(n_index, minlength=B), 1.0)
    pooled = np.zeros((B, H), np.float64)
    np.add.at(pooled, n_index, h6_full_real.astype(np.float64))
    g_mean = (pooled / cnt[:, None]).astype(np.float32)
    return g_mean @ np.asarray(head_w, np.float32) + np.asarray(head_b, np.float32)


def host_input_aggregates(inp, st):
    """Exact f32 input-stage aggregation per core -> [NC][64, TR] bf16 (transposed)."""
    src = np.asarray(inp["src"]).astype(np.int64)
    dst = np.asarray(inp["dst"]).astype(np.int64)
    w_e = (st["no"][src] * st["ni"][dst]).astype(np.float32)
    nx = np.asarray(inp["node_x"], np.float32)
    agg_x = np.zeros((N, NF), np.float32)
    np.add.at(agg_x, dst, w_e[:, None] * nx[src])
    ratio = st["ratio"]
    ee = np.asarray(inp["edge_e"], np.float32) * ratio[:, None]
    e_aug = np.concatenate([ee, ratio[:, None]], 1)
    agg_e = np.zeros((N, 17), np.float32)
    np.add.at(agg_e, dst, w_e[:, None] * e_aug)
    outs = []
    for k in range(NC):
        a = np.zeros((64, TR), np.float32)
        a[0:NF, :RPC] = agg_x[k * RPC:(k + 1) * RPC].T
        a[32:49, :RPC] = agg_e[k * RPC:(k + 1) * RPC].T
        outs.append(a.astype(bf16))
    return outs


def kernel(**inputs):
    out, _res, _h6 = run(inputs, trace=False)
    return out.astype(np.float32)


dt = mybir.dt
AF = mybir.ActivationFunctionType
ALU = mybir.AluOpType
NCORES = NC
MAGIC = 0x5f3759df


def build(st, nlayers=6):
    ntot = st["ntot"]
    tile_w = st["tile_w"]
    call_list = st["call_list"]
    s_off = st["s_off"]
    s_cols = st["s_cols"]
    tlo = st["tspan_lo"]
    thi = st["tspan_hi"]

    nc = bacc.Bacc("TRN2", target_bir_lowering=False, debug=False,
                   enable_asserts=False, num_devices=NCORES, num_swdge_queues=4)

    # ---------- I/O ----------
    S_in = nc.dram_tensor("S_in", [128, s_cols], dt.float8e4, kind="ExternalInput").ap()
    gidx = nc.dram_tensor("gidx", [128, ntot * 8], dt.int16, kind="ExternalInput").ap()
    nidx = nc.dram_tensor("nidx", [128, NT * 8], dt.int16, kind="ExternalInput").ap()
    embT = nc.dram_tensor("embT", [128, B], dt.bfloat16, kind="ExternalInput").ap()
    w1 = nc.dram_tensor("w1", [128, 256], dt.bfloat16, kind="ExternalInput").ap()
    w2 = nc.dram_tensor("w2", [256, 256], dt.bfloat16, kind="ExternalInput").ap()
    tb1 = nc.dram_tensor("tb1", [128, 2], dt.float32, kind="ExternalInput").ap()
    tb2 = nc.dram_tensor("tb2", [128, 2], dt.float32, kind="ExternalInput").ap()
    film = nc.dram_tensor("film", [6 * 257, 512], dt.bfloat16, kind="ExternalInput").ap()
    wconv = nc.dram_tensor("wconv", [6 * 256, 256], dt.bfloat16, kind="ExternalInput").ap()
    bconv = nc.dram_tensor("bconv", [6, 256], dt.bfloat16, kind="ExternalInput").ap()
    win_w = nc.dram_tensor("win_w", [32, 256], dt.bfloat16, kind="ExternalInput").ap()
    we_w = nc.dram_tensor("we_w", [32, 256], dt.bfloat16, kind="ExternalInput").ap()
    inb = nc.dram_tensor("inb", [1, 256], dt.bfloat16, kind="ExternalInput").ap()
    axe = nc.dram_tensor("axe", [64, TR], dt.bfloat16, kind="ExternalInput").ap()
    h6_out = nc.dram_tensor("h6_out", [TR, 256], dt.float32, kind="ExternalOutput").ap()

    # ---------- internal DRAM ----------
    agin = [nc.dram_tensor(f"agin{i}", [TR, 256], dt.bfloat16, kind="Internal").ap()
            for i in range(2)]
    tabs = [nc.dram_tensor(f"tab{i}", [TROWS, 256], dt.bfloat16, kind="Internal",
                           addr_space="Shared").ap() for i in range(2)]
    gbd = nc.dram_tensor("gbd", [6 * B, 512], dt.bfloat16, kind="Internal").ap()

    RG = [list(range(NCORES))]

    def windows_of():
        out = []
        ci = 0
        for w in range(NW):
            ts = list(range(w * WLEN, min((w + 1) * WLEN, NT)))
            calls = []
            while ci < len(call_list):
                b, a, n = call_list[ci]
                if tile_w[a] != w:
                    break
                calls.append((b, a, n))
                ci += 1
            out.append((w, ts, calls))
        return out

    WINS = windows_of()
    W_MAX = max(sum(n for (_, _, n) in calls) for (_, _, calls) in WINS)
    SCW_MAX = 0
    for (w, ts, calls) in WINS:
        a0 = calls[0][1]
        a1 = calls[-1][1] + calls[-1][2] - 1
        w_sc = int(s_off[a1] + (thi[a1] - tlo[a1] + 1) * 128 - s_off[a0])
        SCW_MAX = max(SCW_MAX, w_sc)
    # chunk id of a tile
    def chunk_of(t):
        for c in range(4):
            if t < CHUNK_T[c + 1]:
                return c
        return 3

    def ag_chunk(nc, src, dstb, c):
        # chunk-major table: AG chunk c writes the contiguous region
        # [NC*r0, NC*r1) of the table (concat over cores of chunk-c rows).
        r0, r1 = CHUNK_T[c] * 128, CHUNK_T[c + 1] * 128
        in_ap = src[r0:r1, :]
        out_ap = dstb[NCORES * r0:NCORES * r1, :]
        nc.gpsimd.collective_compute(
            "AllGather", ALU.bypass, replica_groups=RG,
            ins=[in_ap.opt()], outs=[out_ap.opt()])

    with tile.TileContext(nc) as tc:
        with tc.tile_pool(name="const", bufs=1) as constp, \
             tc.tile_pool(name="resp", bufs=1) as resp, \
             tc.tile_pool(name="wpool", bufs=2) as wpool, \
             tc.tile_pool(name="gpool", bufs=2) as gpool, \
             tc.tile_pool(name="spool", bufs=2) as spool, \
             tc.tile_pool(name="gbpool", bufs=3) as gbpool, \
             tc.tile_pool(name="agst", bufs=2) as agstp, \
             tc.tile_pool(name="work", bufs=3) as work, \
             tc.tile_pool(name="tiny", bufs=3) as tiny, \
             tc.tile_pool(name="psZ", bufs=5, space="PSUM") as psZ:

            # ---- resident constants ----
            gidx_sb = constp.tile([128, ntot * 8], dt.int16)
            nc.sync.dma_start(gidx_sb[:], gidx[:])
            nidx_sb = constp.tile([128, NT * 8], dt.int16)
            nc.sync.dma_start(nidx_sb[:], nidx[:])
            ones1 = constp.tile([1, 128], dt.bfloat16)
            nc.vector.memset(ones1[:], 1.0)
            win_sb = constp.tile([32, 256], dt.bfloat16)
            nc.sync.dma_start(win_sb[:], win_w[:])
            we_sb = constp.tile([32, 256], dt.bfloat16)
            nc.sync.dma_start(we_sb[:], we_w[:])
            inb_sb = constp.tile([1, 256], dt.bfloat16)
            nc.sync.dma_start(inb_sb[:], inb[:])

            # residual state, fp32, resident in SBUF
            res = resp.tile([128, NT, 256], dt.float32)

            # ---- phase B: input stage (aggregates precomputed on host) ----
            with tc.tile_pool(name="axp", bufs=2) as axp:
                for g in range(NGRP):
                    t0g = g * GRP
                    ntl = min(GRP, NT - t0g)
                    ax_sb = axp.tile([32, GRP * 128], dt.bfloat16, tag="axw")
                    nc.sync.dma_start(ax_sb[:, 0:ntl * 128],
                                      axe[0:32, t0g * 128:(t0g + ntl) * 128])
                    ae_sb = axp.tile([32, GRP * 128], dt.bfloat16, tag="aew")
                    nc.sync.dma_start(ae_sb[:, 0:ntl * 128],
                                      axe[32:64, t0g * 128:(t0g + ntl) * 128])
                    ag = agstp.tile([128, GRP, 256], dt.bfloat16, tag="ag")
                    for i in range(ntl):
                        t = t0g + i
                        zps = psZ.tile([128, 256], dt.float32, tag="z", space="PSUM")
                        nc.tensor.matmul(zps[:], ax_sb[:, i * 128:(i + 1) * 128],
                                         win_sb[:32, :], start=True, stop=False)
                        nc.tensor.matmul(zps[:], ae_sb[:, i * 128:(i + 1) * 128],
                                         we_sb[:32, :], start=False, stop=False)
                        nc.tensor.matmul(zps[:], ones1[:], inb_sb[:], start=False, stop=True)
                        nc.scalar.activation(res[:, t, :], zps[:], AF.Copy)
                        nc.vector.tensor_copy(ag[:, i, :], zps[:])
                    nc.sync.dma_start(
                        agin[0].rearrange("(t p) f -> p t f", p=128)[:, t0g:t0g + ntl, :],
                        ag[:, 0:ntl, :])
                    # fire AG chunks as their groups complete
                    for c in range(4):
                        if CHUNK_T[c + 1] == t0g + ntl:
                            ag_chunk(nc, agin[0], tabs[0], c)

            # ---- phase A: cond -> gamma/beta tables (overlaps phase B DMA) ----
            with tc.tile_pool(name="condp", bufs=1) as condp, \
                 tc.tile_pool(name="psC", bufs=2, space="PSUM") as psC:
                embT_sb = condp.tile([128, B], dt.bfloat16)
                nc.sync.dma_start(embT_sb[:], embT[:])
                w1_sb = condp.tile([128, 256], dt.bfloat16)
                nc.sync.dma_start(w1_sb[:], w1[:])
                w2_sb = condp.tile([128, 2, 256], dt.bfloat16)
                nc.sync.dma_start(w2_sb[:], w2[:].rearrange("(a p) n -> p a n", p=128))
                tb1_sb = condp.tile([128, 2], dt.float32)
                nc.sync.dma_start(tb1_sb[:], tb1[:])
                tb2_sb = condp.tile([128, 2], dt.float32)
                nc.sync.dma_start(tb2_sb[:], tb2[:])

                c1T = condp.tile([128, 2, B], dt.bfloat16)  # [part, oc, t]
                for oc in range(2):
                    for tb in range(2):
                        ps = psC.tile([128, 512], dt.float32, tag="condps", space="PSUM")
                        nc.tensor.matmul(ps[:], w1_sb[:, oc * 128:(oc + 1) * 128],
                                         embT_sb[:, tb * 512:(tb + 1) * 512],
                                         start=True, stop=True)
                        nc.scalar.activation(c1T[:, oc, tb * 512:(tb + 1) * 512], ps[:],
                                             AF.Silu, bias=tb1_sb[:, oc:oc + 1])
                c2T = condp.tile([128, 2, B], dt.bfloat16)
                for oc in range(2):
                    for tb in range(2):
                        ps = psC.tile([128, 512], dt.float32, tag="condps", space="PSUM")
                        for k in range(2):
                            nc.tensor.matmul(ps[:], w2_sb[:, k, oc * 128:(oc + 1) * 128],
                                             c1T[:, k, tb * 512:(tb + 1) * 512],
                                             start=(k == 0), stop=(k == 1))
                        nc.scalar.activation(c2T[:, oc, tb * 512:(tb + 1) * 512], ps[:],
                                             AF.Identity, bias=tb2_sb[:, oc:oc + 1])
                for l in range(6):
                    film_sb = wpool.tile([128, 2, 512], dt.bfloat16, tag="film")
                    nc.sync.dma_start(film_sb[:], film[l * 257:l * 257 + 256, :]
                                      .rearrange("(a p) n -> p a n", p=128))
                    filmb_sb = wpool.tile([1, 512], dt.bfloat16, tag="filmb")
                    nc.sync.dma_start(filmb_sb[:], film[l * 257 + 256:l * 257 + 257, :])
                    for gc in range(8):
                        ps = psC.tile([128, 512], dt.float32, tag="condps", space="PSUM")
                        for k in range(2):
                            nc.tensor.matmul(ps[:], c2T[:, k, gc * 128:(gc + 1) * 128],
                                             film_sb[:, k, :], start=(k == 0), stop=False)
                        nc.tensor.matmul(ps[:], ones1[:], filmb_sb[:],
                                         start=False, stop=True)
                        gbt = tiny.tile([128, 512], dt.bfloat16, tag="gbt")
                        nc.vector.tensor_copy(gbt[:], ps[:])
                        nc.sync.dma_start(
                            gbd[(l * B + gc * 128):(l * B + (gc + 1) * 128), :], gbt[:])

            # ---- phase C: layers ----
            with tc.tile_pool(name="psAgg", bufs=1, space="PSUM") as psAgg:
                for l in range(nlayers):
                    table = tabs[l % 2]
                    last = l == nlayers - 1

                    wl_sb = wpool.tile([128, 2, 256], dt.bfloat16, tag="wl")
                    nc.sync.dma_start(wl_sb[:], wconv[l * 256:(l + 1) * 256, :]
                                      .rearrange("(a p) n -> p a n", p=128))
                    bl_sb = wpool.tile([1, 256], dt.bfloat16, tag="bl")
                    nc.sync.dma_start(bl_sb[:], bconv[l:l + 1, :])

                    gb_tiles = {}

                    def gb_fetch(wi):
                        if wi >= NW:
                            return
                        t0c = wi * WLEN
                        ntl = min(WLEN, NT - t0c)
                        gbg = gbpool.tile([128, 4, 512], dt.bfloat16, tag="gbg")
                        nc.gpsimd.dma_gather(
                            gbg[:, 0:ntl, :], gbd[l * B:(l + 1) * B, :],
                            nidx_sb[:, t0c * 8:(t0c + ntl) * 8], ntl * 128, ntl * 128, 512,
                            queue_num=wi % 4)
                        for i in range(ntl):
                            gb_tiles[t0c + i] = (gbg, i)

                    gb_fetch(0)
                    gb_fetch(1)
                    qi = 0
                    ag = None
                    for (w, ts, calls) in WINS:
                        alo = psAgg.tile([128, 512], dt.float32, tag="alo", space="PSUM")
                        ahi = psAgg.tile([128, 512], dt.float32, tag="ahi", space="PSUM")
                        w_first = calls[0][1]
                        w_last = calls[-1][1] + calls[-1][2] - 1
                        ws0 = int(s_off[w_first])
                        w_sc = int(s_off[w_last] + (thi[w_last] - tlo[w_last] + 1) * 128) - ws0
                        s_sb = spool.tile([128, SCW_MAX], dt.float8e4, tag="s")
                        nc.sync.dma_start(s_sb[:, 0:w_sc], S_in[:, ws0:ws0 + w_sc])
                        g_win = gpool.tile([128, W_MAX, 256], dt.bfloat16, tag="g")
                        off = 0
                        for (b, a, n) in calls:
                            nc.gpsimd.dma_gather(
                                g_win[:, off:off + n, :], table[b * BANK:(b + 1) * BANK, :],
                                gidx_sb[:, a * 8:(a + n) * 8], n * 128, n * 128, 256,
                                queue_num=qi % 4)
                            qi += 1
                            for i in range(n):
                                ti = a + i
                                lo = int(tlo[ti])
                                width = (int(thi[ti]) - lo + 1) * 128
                                sc0 = int(s_off[ti]) - ws0
                                st_f = ti == w_first
                                sp_f = ti == w_last
                                nc.tensor.matmul(alo[:, lo * 128:lo * 128 + width],
                                                 g_win[:, off + i, 0:128],
                                                 s_sb[:, sc0:sc0 + width],
                                                 start=st_f, stop=sp_f)
                                nc.tensor.matmul(ahi[:, lo * 128:lo * 128 + width],
                                                 g_win[:, off + i, 128:256],
                                                 s_sb[:, sc0:sc0 + width],
                                                 start=st_f, stop=sp_f)
                            off += n
                        gb_fetch(w + 2)
                        # post-process the window's 4 dst tiles
                        zpss = []
                        st6 = tiny.tile([128, WLEN * 6], dt.float32, tag="st6")
                        mvt = tiny.tile([128, WLEN * 2], dt.float32, tag="mvt")
                        for j, t in enumerate(ts):
                            seg = t % WLEN
                            ab_lo = work.tile([128, 128], dt.bfloat16, tag="ablo")
                            nc.scalar.activation(ab_lo[:], alo[:, seg * 128:(seg + 1) * 128],
                                                 AF.Copy)
                            ab_hi = work.tile([128, 128], dt.bfloat16, tag="abhi")
                            nc.scalar.activation(ab_hi[:], ahi[:, seg * 128:(seg + 1) * 128],
                                                 AF.Copy)
                            zps = psZ.tile([128, 256], dt.float32, tag="z", space="PSUM")
                            nc.tensor.matmul(zps[:], ab_lo[:], wl_sb[:, 0, :], start=True, stop=False)
                            nc.tensor.matmul(zps[:], ab_hi[:], wl_sb[:, 1, :], start=False, stop=False)
                            nc.tensor.matmul(zps[:], ones1[:], bl_sb[:], start=False, stop=True)
                            zpss.append(zps)
                            nc.vector.bn_stats(st6[:, j * 6:(j + 1) * 6], zps[:])
                            nc.vector.bn_aggr(mvt[:, j * 2:(j + 1) * 2], st6[:, j * 6:(j + 1) * 6])
                        # full-tile LN scale: rsqrt(var+eps) via bit-trick + 2 Newton
                        # iterations run on the interleaved (mean,var) tile; the
                        # rsqrt of the mean columns is computed but unused.
                        vepst = tiny.tile([128, WLEN * 2], dt.float32, tag="vepst")
                        nc.vector.tensor_scalar_add(vepst[:], mvt[:], LN_EPS)
                        negmvt = tiny.tile([128, WLEN * 2], dt.float32, tag="negmvt")
                        nc.vector.tensor_scalar_mul(negmvt[:], mvt[:], -1.0)
                        yi = tiny.tile([128, WLEN * 2], dt.int32, tag="yi")
                        nc.vector.tensor_scalar(out=yi[:], in0=vepst[:].bitcast(dt.int32),
                                                scalar1=1, scalar2=None,
                                                op0=ALU.arith_shift_right)
                        nc.vector.tensor_scalar(out=yi[:], in0=yi[:],
                                                scalar1=-1, scalar2=MAGIC,
                                                op0=ALU.mult, op1=ALU.add)
                        y0 = yi[:].bitcast(dt.float32)
                        rstd = tiny.tile([128, WLEN * 2], dt.float32, tag="rstd")
                        tnw = tiny.tile([128, WLEN * 2], dt.float32, tag="tnw")
                        # two Newton iterations: y = y*(1.5 - 0.5*v*y*y)
                        nc.vector.tensor_tensor(out=tnw[:], in0=vepst[:], in1=y0, op=ALU.mult)
                        nc.vector.tensor_tensor(out=tnw[:], in0=tnw[:], in1=y0, op=ALU.mult)
                        nc.vector.tensor_scalar(out=tnw[:], in0=tnw[:],
                                                scalar1=-0.5, scalar2=1.5, op0=ALU.mult, op1=ALU.add)
                        nc.vector.tensor_tensor(out=rstd[:], in0=y0, in1=tnw[:], op=ALU.mult)
                        nc.vector.tensor_tensor(out=tnw[:], in0=vepst[:], in1=rstd[:], op=ALU.mult)
                        nc.vector.tensor_tensor(out=tnw[:], in0=tnw[:], in1=rstd[:], op=ALU.mult)
                        nc.vector.tensor_scalar(out=tnw[:], in0=tnw[:],
                                                scalar1=-0.5, scalar2=1.5, op0=ALU.mult, op1=ALU.add)
                        nc.vector.tensor_tensor(out=rstd[:], in0=rstd[:], in1=tnw[:], op=ALU.mult)
                        nc.vector.tensor_tensor(out=tnw[:], in0=vepst[:], in1=rstd[:], op=ALU.mult)
                        nc.vector.tensor_tensor(out=tnw[:], in0=tnw[:], in1=rstd[:], op=ALU.mult)
                        nc.vector.tensor_scalar(out=tnw[:], in0=tnw[:],
                                                scalar1=-0.5, scalar2=1.5, op0=ALU.mult, op1=ALU.add)
                        nc.vector.tensor_tensor(out=rstd[:], in0=rstd[:], in1=tnw[:], op=ALU.mult)

                        for j, t in enumerate(ts):
                            zps = zpss[j]
                            xh = work.tile([128, 256], dt.bfloat16, tag="xh")
                            nc.vector.tensor_scalar(out=xh[:], in0=zps[:],
                                                    scalar1=negmvt[:, 2 * j:2 * j + 1],
                                                    scalar2=rstd[:, 2 * j + 1:2 * j + 2],
                                                    op0=ALU.add, op1=ALU.mult)
                            gbg, gi = gb_tiles[t]
                            y = work.tile([128, 256], dt.bfloat16, tag="y")
                            nc.vector.tensor_tensor(out=y[:], in0=xh[:], in1=gbg[:, gi, 0:256], op=ALU.mult)
                            nc.vector.tensor_tensor(out=y[:], in0=y[:], in1=gbg[:, gi, 256:512], op=ALU.add)
                            h2 = work.tile([128, 256], dt.bfloat16, tag="h2")
                            nc.scalar.activation(h2[:], y[:], AF.Silu)
                            nc.vector.tensor_tensor(out=res[:, t, :], in0=h2[:],
                                                    in1=res[:, t, :], op=ALU.add)
                            if not last:
                                ig = t - (t // GRP) * GRP
                                if ig == 0:
                                    ag = agstp.tile([128, GRP, 256], dt.bfloat16, tag="ag")
                                nc.scalar.activation(ag[:, ig, :], res[:, t, :], AF.Copy)
                                if t == NT - 1 or ig == GRP - 1:
                                    t0g = (t // GRP) * GRP
                                    ntl = t - t0g + 1
                                    nc.sync.dma_start(
                                        agin[(l + 1) % 2].rearrange(
                                            "(t p) f -> p t f", p=128)[:, t0g:t0g + ntl, :],
                                        ag[:, 0:ntl, :])
                                for c in range(4):
                                    if CHUNK_T[c + 1] == t + 1:
                                        ag_chunk(nc, agin[(l + 1) % 2], tabs[(l + 1) % 2], c)

                # final output: residual SBUF -> DRAM
                nc.sync.dma_start(
                    h6_out.rearrange("(t p) f -> p t f", p=128), res[:, :, :])

    nc.compile()
    return nc


def make_inputs(inp, st):
    """Build per-core in_maps."""
    n_index = np.asarray(inp["n_index"]).astype(np.int64)
    ntot = st["ntot"]

    emb = sincos_emb(inp["t"])
    embT = np.ascontiguousarray(emb.T).astype(bf16)

    film_tab = prep_weights(inp)
    film_dev = film_tab.reshape(6 * 257, 512).astype(bf16)
    wconv = np.asarray(inp["conv2_w"], np.float32).reshape(6 * 256, 256).astype(bf16)
    bconv = np.asarray(inp["conv2_b"], np.float32).astype(bf16)

    w_in32 = np.zeros((32, 256), bf16)
    w_in32[:NF] = np.asarray(inp["in_conv_w"], np.float32).astype(bf16)
    we32 = np.zeros((32, 256), bf16)
    we32[:EF] = np.asarray(inp["edge_w"], np.float32).astype(bf16)
    we32[EF] = np.asarray(inp["edge_b"], np.float32).astype(bf16)
    inb = np.asarray(inp["in_conv_b"], np.float32).astype(bf16)[None, :]

    tb1 = np.ascontiguousarray(np.asarray(inp["t_b1"], np.float32).reshape(2, 128).T)
    tb2 = np.ascontiguousarray(np.asarray(inp["t_b2"], np.float32).reshape(2, 128).T)
    w1 = np.asarray(inp["t_w1"], np.float32).astype(bf16)
    w2 = np.asarray(inp["t_w2"], np.float32).astype(bf16)

    axes = host_input_aggregates(inp, st)
    S = build_S(st)

    in_maps = []
    for k in range(NCORES):
        S_dev = S[k]
        gidx_dev = wrap_idx(st["gidx"][k])
        gvals = np.zeros(TR, np.int16)
        gvals[:RPC] = n_index[k * RPC:(k + 1) * RPC].astype(np.int16)
        nidx_dev = wrap_idx(gvals.reshape(NT, 128))
        in_maps.append({
            "S_in": S_dev, "gidx": gidx_dev,
            "nidx": nidx_dev, "embT": embT, "w1": w1, "w2": w2,
            "tb1": tb1, "tb2": tb2, "film": film_dev, "wconv": wconv,
            "bconv": bconv, "win_w": w_in32, "we_w": we32, "inb": inb,
            "axe": axes[k],
        })
    return in_maps


def run(inp, trace=False, nlayers=6):
    src = np.asarray(inp["src"]).astype(np.int64)
    dst = np.asarray(inp["dst"]).astype(np.int64)
    st = build_edge_structure(src, dst)
    nc = build(st, nlayers=nlayers)
    in_maps = make_inputs(inp, st)
    res = bass_utils.run_bass_kernel_spmd(
        nc, in_maps, core_ids=list(range(NCORES)), trace=trace,
        trace_cores=[0] if trace else None)
    h6 = np.concatenate([res.results[k]["h6_out"][:RPC] for k in range(NCORES)], 0)
    out = host_pool_head(h6, np.asarray(inp["n_index"]), inp["head_w"], inp["head_b"])
    return out, res, h6
